# revision 1
# baseline (speedup 1.0000x reference)
"""Modulated deformable conv v2 (B=8, C=O=256, H=W=64, 3x3) on 8 trn2 NeuronCores.

Strategy: data-parallel over batch (1 image per core). Per core:
  - host marshals the image into a "patch array" in3[j] = concat of padded
    [HW, C] rows (j, j+1, j+64, j+65) so that one contiguous 4KB DMA descriptor
    fetches the full 2x2 bilinear patch for all 256 channels of one sample.
  - device computes patch indices (floor/clip of offsets) and the 4 folded
    corner weights (bilinear frac * validity * modulation mask) on DVE.
  - gpsimd dma_gather streams patch rows (HBM->SBUF, ~360GB/s), DVE applies
    the 4 per-row corner weights, PE transposes [p,c]->[c,p] via identity
    matmuls, ACT copies PSUM->SBUF, PE runs the O x (C*K2) x HW einsum with
    PSUM accumulation, ACT folds the bias, HWDGE DMAs the output back.
"""

import os
import numpy as np
from contextlib import ExitStack

import concourse.bacc as bacc
import concourse.bass as bass
import concourse.mybir as mybir
from concourse import bass_utils
from concourse.library_config import mlp

AP = bass.AP
F32 = mybir.dt.float32
I16 = mybir.dt.int16

# problem constants (hardcoded per contract)
B = 8
C = 256
O = 256
H = W = 64
HW = 4096
K2 = 9

# tiling
NCH = 8          # spatial chunks
CHP = 512        # positions per chunk
NG = 4           # 128-position groups per chunk
NITER = NCH * K2 # 72 macro iterations (chunk-major, then k)
NGI = NITER * NG # 288 (i,g) steps

PADLO = 65       # leading pad rows in the padded [HW, C] image
R2 = 4292        # padded image rows
R3 = 4232        # patch-array rows (4225 used)
MAGIC = 12582912.0  # 1.5 * 2**23, for round-to-nearest-even on f32

DT_NAME = os.environ.get("DEFORM_DT", "f32")


def _dt():
    return mybir.dt.bfloat16 if DT_NAME == "bf16" else mybir.dt.float32


def _np_dt():
    if DT_NAME == "bf16":
        import ml_dtypes
        return ml_dtypes.bfloat16
    return np.float32


# ---------------------------------------------------------------------------
# bass program
# ---------------------------------------------------------------------------

def build_nc():
    dt = _dt()
    nc = bacc.Bacc("TRN2", detect_race_conditions=False)

    in3 = nc.dram_tensor("in3", [R3, 1024], dt, kind="ExternalInput")
    wT = nc.dram_tensor("wT", [2304, 256], dt, kind="ExternalInput")
    identm = nc.dram_tensor("identm", [128, 128], dt, kind="ExternalInput")
    biasm = nc.dram_tensor("biasm", [128, 2], F32, kind="ExternalInput")
    dyW = nc.dram_tensor("dyW", [128, 288], F32, kind="ExternalInput")
    dxW = nc.dram_tensor("dxW", [128, 288], F32, kind="ExternalInput")
    mW = nc.dram_tensor("mW", [128, 288], F32, kind="ExternalInput")
    byW = nc.dram_tensor("byW", [128, 288], F32, kind="ExternalInput")
    bxW = nc.dram_tensor("bxW", [128, 288], F32, kind="ExternalInput")
    dyI = nc.dram_tensor("dyI", [128, 2304], F32, kind="ExternalInput")
    dxI = nc.dram_tensor("dxI", [128, 2304], F32, kind="ExternalInput")
    byI = nc.dram_tensor("byI", [128, 2304], F32, kind="ExternalInput")
    bxI = nc.dram_tensor("bxI", [128, 2304], F32, kind="ExternalInput")
    fence_dram = nc.dram_tensor("fenced", [128, 8], F32, kind="ExternalInput")
    outT = nc.dram_tensor("out", [256, 4096], F32, kind="ExternalOutput")
    DEBUG = os.environ.get("DEFORM_DEBUG") == "1"
    if DEBUG:
        dbg_jw = nc.dram_tensor("dbg_jw", [128, 2304], I16, kind="ExternalOutput")
        dbg_wc = nc.dram_tensor("dbg_wc", [128, 4 * 288], F32, kind="ExternalOutput")
        dbg_gd = nc.dram_tensor("dbg_gd", [128, 4096], F32, kind="ExternalOutput")
        dbg_sr = nc.dram_tensor("dbg_sr", [128, 1024], F32, kind="ExternalOutput")
        dbg_st = nc.dram_tensor("dbg_st", [128, 512], F32, kind="ExternalOutput")
        dbg_s4 = nc.dram_tensor("dbg_s4", [128, 256], F32, kind="ExternalOutput")

    with ExitStack() as ctx:
        ec = ctx.enter_context

        # sbuf
        gdst = [ec(nc.sbuf_tensor(f"gdst{j}", [128, 4096], dt)) for j in range(2)]
        s_rows = [ec(nc.sbuf_tensor(f"srows{j}", [128, 1024], dt)) for j in range(2)]
        st_sb = [ec(nc.sbuf_tensor(f"stsb{j}", [128, 256], dt)) for j in range(2)]
        out_sb = [ec(nc.sbuf_tensor(f"outsb{j}", [128, 1024], F32)) for j in range(2)]
        w_sb = ec(nc.sbuf_tensor("wsb", [128, 4608], dt))
        ident_sb = ec(nc.sbuf_tensor("identsb", [128, 128], dt))
        bias_sb = ec(nc.sbuf_tensor("biassb", [128, 2], F32))
        tmpA = [ec(nc.sbuf_tensor(f"tmpA{g}", [128, 256], dt)) for g in range(NG)]
        tmpB = [ec(nc.sbuf_tensor(f"tmpB{g}", [128, 256], dt)) for g in range(NG)]

        # W-layout field tiles [128, 288]
        fW_in = {n: ec(nc.sbuf_tensor(f"f_{n}", [128, 288], F32))
                 for n in ("dyW", "dxW", "mW", "byW", "bxW")}
        wcor = [ec(nc.sbuf_tensor(f"wc{q}", [128, 288], F32)) for q in range(4)]
        tw = [ec(nc.sbuf_tensor(f"tw{j}", [128, 288], F32)) for j in range(7)]

        # I-layout tiles [128, 2304]
        fI_in = {n: ec(nc.sbuf_tensor(f"f_{n}", [128, 2304], F32))
                 for n in ("dyI", "dxI", "byI", "bxI")}
        ti = [ec(nc.sbuf_tensor(f"ti{j}", [128, 2304], F32)) for j in range(4)]
        jw = ec(nc.sbuf_tensor("jw", [128, 2304], I16))

        # psum: full banks to avoid bank sharing
        bank_elems = 512 if dt == F32 else 1024
        psT = [ec(nc.psum_tensor(f"psT{j}", [128, bank_elems], dt)) for j in range(2)]
        psE = [ec(nc.psum_tensor(f"psE{g}", [128, 512], F32)) for g in range(NG)]

        sem_ld = ec(nc.semaphore("sem_ld"))
        sem_prep = ec(nc.semaphore("sem_prep"))
        sem_prep2 = ec(nc.semaphore("sem_prep2"))
        sem_gat = ec(nc.semaphore("sem_gat"))
        sem_dve = ec(nc.semaphore("sem_dve"))
        sem_pet = ec(nc.semaphore("sem_pet"))
        sem_act = ec(nc.semaphore("sem_act"))
        sem_pee = ec(nc.semaphore("sem_pee"))
        sem_epi = ec(nc.semaphore("sem_epi"))
        sem_out = ec(nc.semaphore("sem_out"))
        sem_fence = ec(nc.semaphore("sem_fence"))
        sem_dbg = ec(nc.semaphore("sem_dbg"))
        fence_sb = ec(nc.sbuf_tensor("fence", [128, 8 * NITER], F32))

        # ---- AP helpers (flat element offsets) ----
        def sb(t, off, free, count=128, pstep=None):
            if pstep is None:
                pstep = t.shape[1] if len(t.shape) == 2 else int(np.prod(t.shape[1:]))
            return AP(t, off, [[pstep, count], [1, free]])

        def scl(t, col):
            return AP(t, col, [[t.shape[1], 128], [1, 1]])

        loads = [
            (sb(ident_sb, 0, 128), AP(identm, 0, [[128, 128], [1, 128]])),
            (sb(bias_sb, 0, 2), AP(biasm, 0, [[2, 128], [1, 2]])),
        ]
        for n, src in (("dyW", dyW), ("dxW", dxW), ("mW", mW), ("byW", byW),
                       ("bxW", bxW)):
            loads.append((sb(fW_in[n], 0, 288), AP(src, 0, [[288, 128], [1, 288]])))
        for n, src in (("dyI", dyI), ("dxI", dxI), ("byI", byI), ("bxI", bxI)):
            loads.append((sb(fI_in[n], 0, 2304), AP(src, 0, [[2304, 128], [1, 2304]])))
        for kcb in range(18):
            loads.append((sb(w_sb, kcb * 256, 256),
                          AP(wT, kcb * 128 * 256, [[256, 128], [1, 256]])))
        n_loads = len(loads)

        glast = {}  # chunk -> gi of last transpose-copy step
        for c in range(NCH):
            glast[(c * K2 + (K2 - 1)) * NG + (NG - 1)] = c

        with nc.Block() as block:

            @block.sync
            def _(sync):
                for dst, src in loads:
                    sync.dma_start(dst, src).then_inc(sem_ld, 16)
                if DEBUG:
                    sync.wait_ge(sem_dve, 4)     # iter0 weighted
                    sync.dma_start(AP(dbg_jw, 0, [[2304, 128], [1, 2304]]),
                                   sb(jw, 0, 2304)).then_inc(sem_dbg, 16)
                    for q in range(4):
                        sync.dma_start(AP(dbg_wc, q * 288, [[1152, 128], [1, 288]]),
                                       sb(wcor[q], 0, 288)).then_inc(sem_dbg, 16)
                    sync.dma_start(AP(dbg_gd, 0, [[4096, 128], [1, 4096]]),
                                   sb(gdst[0], 0, 4096)).then_inc(sem_dbg, 16)
                    sync.dma_start(AP(dbg_sr, 0, [[1024, 128], [1, 1024]]),
                                   sb(s_rows[0], 0, 1024)).then_inc(sem_dbg, 16)
                    sync.wait_ge(sem_act, 2)     # gi=0,1 copied
                    sync.dma_start(AP(dbg_st, 0, [[512, 128], [1, 256]]),
                                   sb(st_sb[0], 0, 256)).then_inc(sem_dbg, 16)
                    sync.dma_start(AP(dbg_st, 256, [[512, 128], [1, 256]]),
                                   sb(st_sb[1], 0, 256)).then_inc(sem_dbg, 16)
                    sync.wait_ge(sem_act, 5)
                    sync.dma_start(AP(dbg_s4, 0, [[256, 128], [1, 256]]),
                                   sb(st_sb[0], 0, 256)).then_inc(sem_dbg, 16)
                for c in range(NCH):
                    sync.wait_ge(sem_epi, 8 * (c + 1))
                    for ob in range(2):
                        dst = AP(outT, ob * 128 * 4096 + c * 512,
                                 [[4096, 128], [1, 512]])
                        src = sb(out_sb[c % 2], ob * 512, 512)
                        sync.dma_start(dst, src).then_inc(sem_out, 16)

            @block.gpsimd
            def _(gp):
                gp.load_library(mlp)
                gp.wait_ge(sem_prep, 1)
                in3_ap = AP(in3, 0, [[1024, R3], [1, 1024]])
                for i in range(NITER):
                    c, k = divmod(i, K2)
                    if DEBUG and i == 2:
                        gp.wait_ge(sem_dbg, 16 * 7)
                    if i >= 2:
                        gp.wait_ge(sem_dve, NG * (i - 1))
                    dst = AP(gdst[i % 2], 0, [[4096, 128], [1024, 4], [1, 1024]])
                    idx = AP(jw, (k * NCH + c) * 32, [[2304, 128], [1, 32]])
                    gp.dma_gather(dst, in3_ap, idx, CHP, CHP, 1024,
                                  prepare_only=True, sem=sem_gat).then_inc(
                        sem_prep2, 1)
                    gp.wait_ge(sem_prep2, i + 1)
                    gp.trigger_dma(count=1)

            @block.vector
            def _(v):
                A = mybir.AluOpType
                v.wait_ge(sem_ld, 16 * n_loads)

                class _G:
                    # auto-insert drain on same-engine RAW/WAR/WAW hazards
                    def __init__(self, eng):
                        self.e = eng
                        self.r = set()
                        self.w = set()

                    def drain(self):
                        self.e.drain()
                        self.r.clear()
                        self.w.clear()

                    def _run(self, fn, outs, ins, args, kwargs):
                        on = {o.tensor.name for o in outs}
                        innames = {a.tensor.name for a in ins
                                   if isinstance(a, AP)}
                        if (on & (self.r | self.w)) or (innames & self.w):
                            self.drain()
                        self.r |= innames
                        self.w |= on
                        return fn(*args, **kwargs)

                    def tensor_add(self, o, a, b):
                        return self._run(self.e.tensor_add, [o], [a, b],
                                         (o, a, b), {})

                    def tensor_sub(self, o, a, b):
                        return self._run(self.e.tensor_sub, [o], [a, b],
                                         (o, a, b), {})

                    def tensor_mul(self, o, a, b):
                        return self._run(self.e.tensor_mul, [o], [a, b],
                                         (o, a, b), {})

                    def tensor_tensor(self, o, a, b, op):
                        return self._run(self.e.tensor_tensor, [o], [a, b],
                                         (o, a, b, op), {})

                    def tensor_scalar(self, o, a, s1, s2, op0, op1):
                        return self._run(self.e.tensor_scalar, [o], [a],
                                         (o, a, s1, s2, op0, op1), {})

                    def tensor_single_scalar(self, o, a, s, op):
                        ins = [a] + ([s] if isinstance(s, AP) else [])
                        return self._run(self.e.tensor_single_scalar, [o], ins,
                                         (o, a, s, op), {})

                    def tensor_copy(self, o, a):
                        return self._run(self.e.tensor_copy, [o], [a],
                                         (o, a), {})

                    def scalar_tensor_tensor(self, o, a, s, b, op0, op1):
                        ins = [a, b] + ([s] if isinstance(s, AP) else [])
                        return self._run(self.e.scalar_tensor_tensor, [o], ins,
                                         (o, a, s, b, op0, op1), {})

                vg = _G(v)

                def D():
                    pass

                # ---- index prep (I-layout [128, 2304]) ----
                FI = 2304
                s1, s2, s3, s4 = (sb(t, 0, FI) for t in ti)
                vg.tensor_add(s1, sb(fI_in["dyI"], 0, FI), sb(fI_in["byI"], 0, FI))
                D()
                vg.tensor_scalar(s2, s1, MAGIC, MAGIC, A.add, A.subtract)
                D()
                vg.tensor_tensor(s3, s2, s1, A.is_gt)
                D()
                vg.tensor_sub(s1, s2, s3)                       # ey
                D()
                vg.tensor_scalar(s2, s1, -1.0, 63.0, A.max, A.min)  # py
                vg.tensor_add(s3, sb(fI_in["dxI"], 0, FI), sb(fI_in["bxI"], 0, FI))
                D()
                vg.tensor_scalar(s1, s3, MAGIC, MAGIC, A.add, A.subtract)
                D()
                vg.tensor_tensor(s4, s1, s3, A.is_gt)
                D()
                vg.tensor_sub(s1, s1, s4)                       # ex
                D()
                vg.tensor_scalar(s3, s1, -1.0, 63.0, A.max, A.min)  # px
                D()
                vg.tensor_scalar(s1, s3, float(PADLO), 0.0, A.add, A.add)
                D()
                vg.scalar_tensor_tensor(s4, s2, 64.0, s1, A.mult, A.add)  # J
                D()
                vg.tensor_copy(AP(jw, 0, [[2304, 128], [1, FI]]), s4).then_inc(
                    sem_prep)

                # ---- corner-weight fields (W-layout [128, 288]) ----
                FW = 288
                t1, t2, t3, t4, t5, t6, t7 = (sb(t, 0, FW) for t in tw)
                dy = sb(fW_in["dyW"], 0, FW)
                dx = sb(fW_in["dxW"], 0, FW)
                mm = sb(fW_in["mW"], 0, FW)
                by = sb(fW_in["byW"], 0, FW)
                bx = sb(fW_in["bxW"], 0, FW)

                vg.tensor_add(t1, dy, by)                          # yA
                D()
                vg.tensor_scalar(t2, t1, MAGIC, MAGIC, A.add, A.subtract)
                D()
                vg.tensor_tensor(t3, t2, t1, A.is_gt)
                D()
                vg.tensor_sub(t4, t2, t3)                          # ey
                D()
                vg.tensor_sub(t5, t1, t4)                          # ly
                D()
                vg.tensor_scalar(t1, t5, -1.0, 1.0, A.mult, A.add)  # hy
                vg.tensor_single_scalar(t2, t4, 0.0, A.is_ge)
                vg.tensor_single_scalar(t3, t4, 63.0, A.is_le)
                D()
                vg.tensor_mul(t2, t2, t3)                          # vy0
                vg.tensor_single_scalar(t3, t4, -1.0, A.is_ge)
                vg.tensor_single_scalar(t6, t4, 62.0, A.is_le)
                D()
                vg.tensor_mul(t3, t3, t6)                          # vy1
                D()
                vg.tensor_mul(t2, t2, t1)
                vg.tensor_mul(t3, t3, t5)
                D()
                vg.tensor_mul(t2, t2, mm)                          # wy0m
                vg.tensor_mul(t3, t3, mm)                          # wy1m
                D()

                vg.tensor_add(t1, dx, bx)                          # xA
                D()
                vg.tensor_scalar(t4, t1, MAGIC, MAGIC, A.add, A.subtract)
                D()
                vg.tensor_tensor(t5, t4, t1, A.is_gt)
                D()
                vg.tensor_sub(t4, t4, t5)                          # ex
                D()
                vg.tensor_sub(t5, t1, t4)                          # lx
                D()
                vg.tensor_scalar(t1, t5, -1.0, 1.0, A.mult, A.add)  # hx
                vg.tensor_single_scalar(t6, t4, 0.0, A.is_ge)
                vg.tensor_single_scalar(t7, t4, 63.0, A.is_le)
                D()
                vg.tensor_mul(t6, t6, t7)                          # vx0
                D()
                vg.tensor_mul(t6, t6, t1)                          # cx0 = hx*vx0
                vg.tensor_single_scalar(t7, t4, -1.0, A.is_ge)
                D()
                vg.tensor_single_scalar(t4, t4, 62.0, A.is_le)
                D()
                vg.tensor_mul(t7, t7, t4)                          # vx1
                D()
                vg.tensor_mul(t7, t7, t5)                          # cx1 = lx*vx1
                D()

                vg.tensor_mul(sb(wcor[0], 0, FW), t2, t6)          # w00
                vg.tensor_mul(sb(wcor[1], 0, FW), t2, t7)          # w01
                vg.tensor_mul(sb(wcor[2], 0, FW), t3, t6)          # w10
                vg.tensor_mul(sb(wcor[3], 0, FW), t3, t7)          # w11
                D()

                # ---- per-(i,g) corner-weight application ----
                # 4 independent groups per stage, one drain between stages
                for i in range(NITER):
                    c, k = divmod(i, K2)
                    if DEBUG and i == 2:
                        v.wait_ge(sem_dbg, 16 * 7)
                    v.wait_ge(sem_gat, 16 * (i + 1))
                    if i >= 2:
                        v.wait_ge(sem_pet, NG * (i - 1))
                    cols = [k * 32 + c * NG + g for g in range(NG)]
                    gsl = [[AP(gdst[i % 2], g * 1024 + q * 256,
                               [[4096, 128], [1, 256]]) for q in range(4)]
                           for g in range(NG)]
                    a_ = [sb(t, 0, 256) for t in tmpA]
                    b_ = [sb(t, 0, 256) for t in tmpB]
                    dstS = [sb(s_rows[i % 2], g * 256, 256) for g in range(NG)]
                    for g in range(NG):
                        vg.tensor_single_scalar(a_[g], gsl[g][0],
                                               scl(wcor[0], cols[g]), A.mult)
                    D()
                    for g in range(NG):
                        vg.scalar_tensor_tensor(b_[g], gsl[g][1],
                                               scl(wcor[1], cols[g]), a_[g],
                                               A.mult, A.add)
                    D()
                    for g in range(NG):
                        vg.scalar_tensor_tensor(a_[g], gsl[g][2],
                                               scl(wcor[2], cols[g]), b_[g],
                                               A.mult, A.add)
                    D()
                    for g in range(NG):
                        vg.scalar_tensor_tensor(dstS[g], gsl[g][3],
                                               scl(wcor[3], cols[g]), a_[g],
                                               A.mult, A.add).then_inc(sem_dve)
                    D()

            @block.tensor
            def _(te):
                te.wait_ge(sem_ld, 16 * n_loads)

                def emit_einsum(gg):
                    i2, g2 = divmod(gg, NG)
                    c2, k2 = divmod(i2, K2)
                    te.wait_ge(sem_act, gg + 1)
                    if k2 == 0 and c2 >= 1:
                        # psE[g2] bank reused across chunks; wait for the
                        # previous chunk's epilogue to finish reading it
                        te.wait_ge(sem_epi, 8 * c2)
                    last = None
                    for ob in range(2):
                        for cb in range(2):
                            lhs = AP(w_sb, (k2 * 2 + cb) * 256 + ob * 128,
                                     [[4608, 128], [1, 128]])
                            rhs = AP(st_sb[gg % 2], cb * 128, [[256, 128], [1, 128]])
                            dst = AP(psE[g2], ob * 128, [[512, 128], [1, 128]])
                            last = te.matmul(dst, lhs, rhs,
                                             start=(k2 == 0 and ob == 0
                                                    and cb == 0),
                                             stop=(k2 == K2 - 1 and ob == 1
                                                   and cb == 1))
                    last.then_inc(sem_pee)

                for i in range(NITER):
                    for g in range(NG):
                        gi = NG * i + g
                        te.wait_ge(sem_dve, gi + 1)
                        if gi >= 2:
                            te.wait_ge(sem_act, gi - 1)
                        pg = gi % 2
                        last = None
                        for cb in range(2):
                            src = AP(s_rows[i % 2], g * 256 + cb * 128,
                                     [[1024, 128], [1, 128]])
                            dst = AP(psT[pg], cb * 128,
                                     [[psT[pg].shape[1], 128], [1, 128]])
                            last = te.transpose(dst, src, sb(ident_sb, 0, 128))
                        last.then_inc(sem_pet)
                        if gi >= 1:
                            emit_einsum(gi - 1)
                emit_einsum(NGI - 1)

            @block.scalar
            def _(sc):
                IDENT = mybir.ActivationFunctionType.Identity
                for gi in range(NGI):
                    sc.wait_ge(sem_pet, gi + 1)
                    if DEBUG and gi == 2:
                        sc.wait_ge(sem_dbg, 16 * 7)
                    if DEBUG and gi == 6:
                        sc.wait_ge(sem_dbg, 16 * 10)
                    if gi >= 2:
                        sc.wait_ge(sem_pee, gi - 1)
                    sc.activation(sb(st_sb[gi % 2], 0, 256),
                                  AP(psT[gi % 2], 0,
                                     [[psT[gi % 2].shape[1], 128], [1, 256]]),
                                  IDENT).then_inc(sem_act)
                    c = glast.get(gi)
                    if c is not None:
                        if c >= 2:
                            sc.wait_ge(sem_out, 32 * (c - 1))
                        for g2 in range(NG):
                            sc.wait_ge(sem_pee, (c * K2 + K2 - 1) * NG + g2 + 1)
                            for ob in range(2):
                                sc.activation(
                                    sb(out_sb[c % 2], ob * 512 + g2 * 128, 128),
                                    AP(psE[g2], ob * 128,
                                       [[512, 128], [1, 128]]),
                                    IDENT, bias=scl(bias_sb, ob),
                                ).then_inc(sem_epi)

    nc.compile()
    return nc


# ---------------------------------------------------------------------------
# host marshalling
# ---------------------------------------------------------------------------

def _to_W(f):
    # f [9, 4096] -> [128, 288]; fW[p%128, k*32 + p//128] = f[k, p]
    return np.ascontiguousarray(
        f.reshape(9, 32, 128).transpose(2, 0, 1).reshape(128, 288))


def _to_I(f):
    # f [9, 4096] -> wrapped [128, 2304]; fI[r, (k*8+c)*32+t] = f[k, c*512+t*16+r%16]
    a = f.reshape(9, 8, 32, 16).transpose(3, 0, 1, 2).reshape(16, 2304)
    return np.ascontiguousarray(np.tile(a, (8, 1)))


def marshal(inputs):
    np_dt = _np_dt()
    inp = np.asarray(inputs["input"], np.float32)
    off = np.asarray(inputs["offset"], np.float32)
    msk = np.asarray(inputs["mask"], np.float32)
    wgt = np.asarray(inputs["weight"], np.float32)
    bias = np.asarray(inputs["bias"], np.float32)

    wT = np.ascontiguousarray(
        wgt.reshape(O, C, K2).transpose(2, 1, 0).reshape(2304, 256)).astype(np_dt)
    identm = np.eye(128, dtype=np.float32).astype(np_dt)
    biasm = np.ascontiguousarray(bias.reshape(2, 128).T)

    ho = np.arange(HW, dtype=np.float32) // 64
    wo = np.arange(HW, dtype=np.float32) % 64
    ks = np.arange(K2, dtype=np.float32)
    by = ho[None, :] - 1.0 + (ks // 3)[:, None]
    bx = wo[None, :] - 1.0 + (ks % 3)[:, None]

    shared = {
        "wT": wT, "identm": identm, "biasm": biasm,
        "byW": _to_W(by), "bxW": _to_W(bx),
        "byI": _to_I(by), "bxI": _to_I(bx),
    }

    in_maps = []
    for b in range(B):
        img = inp[b].transpose(1, 2, 0).reshape(HW, C)
        in2p = np.zeros((R2, C), np.float32)
        in2p[PADLO:PADLO + HW] = img
        in3 = np.zeros((R3, 1024), np.float32)
        n = HW + 2 * PADLO - 1  # 4225 usable rows
        in3[:n, 0:256] = in2p[0:n]
        in3[:n, 256:512] = in2p[1:n + 1]
        in3[:n, 512:768] = in2p[64:n + 64]
        in3[:n, 768:1024] = in2p[65:n + 65]

        off_y = np.ascontiguousarray(off[b, 0::2].reshape(K2, HW))
        off_x = np.ascontiguousarray(off[b, 1::2].reshape(K2, HW))
        im = {
            "fenced": np.zeros((128, 8), np.float32),
            "in3": in3.astype(np_dt),
            "dyW": _to_W(off_y), "dxW": _to_W(off_x),
            "mW": _to_W(msk[b].reshape(K2, HW)),
            "dyI": _to_I(off_y), "dxI": _to_I(off_x),
        }
        im.update(shared)
        in_maps.append(im)
    return in_maps


_NC_CACHE = {}


def _get_nc():
    if DT_NAME not in _NC_CACHE:
        _NC_CACHE[DT_NAME] = build_nc()
    return _NC_CACHE[DT_NAME]


def run(inputs, trace=False, **kw):
    nc = _get_nc()
    in_maps = marshal(inputs)
    res = bass_utils.run_bass_kernel_spmd(nc, in_maps, core_ids=list(range(B)),
                                          trace=trace, **kw)
    out = np.stack([r["out"].reshape(O, H, W) for r in res.results])
    return out.astype(np.float32), res


def kernel(**inputs):
    return run(inputs)[0]



# revision 3
# speedup vs baseline: 1.8518x; 1.8518x over previous
"""Modulated deformable conv v2 (B=8, C=O=256, H=W=64, 3x3) on 8 trn2 NeuronCores.

Strategy: data-parallel over batch (1 image per core). Per core:
  - host marshals the image into a "patch array" in3[j] = concat of padded
    [HW, C] rows (j, j+1, j+64, j+65) in bf16 so one contiguous 2KB DMA
    descriptor fetches the full 2x2 bilinear patch for all 256 channels of
    one sample.
  - device computes patch indices (floor/clip of offsets) on DVE, gpsimd
    dma_gather streams patch rows (HBM->SBUF) in 1024-sample blocks.
  - corner weighting + transpose + corner-sum are fused on the PE: for each
    128-position group, 8 matmuls  psT[ch, pos] += gdst_q[pos, ch]^T @
    diag(w_q[pos])  accumulate the 4 bilinear corners (weights fold
    bilinear frac * validity * modulation mask).  The diagonal tiles are
    rebuilt per iteration by one DVE op: Dd = ident16 * broadcast(w).
  - ACT copies PSUM->SBUF (cast bf16), PE runs the O x (C*K2) x HW einsum
    with PSUM accumulation, ACT folds the bias, HWDGE DMAs the output back.
"""

import numpy as np
from contextlib import ExitStack

import concourse.bacc as bacc
import concourse.bass as bass
import concourse.mybir as mybir
from concourse import bass_utils
from concourse.library_config import mlp

AP = bass.AP
F32 = mybir.dt.float32
BF16 = mybir.dt.bfloat16
I16 = mybir.dt.int16

# problem constants (hardcoded per contract)
B = 8
C = 256
O = 256
H = W = 64
HW = 4096
K2 = 9

NCH = 8           # spatial chunks
NG = 4            # 128-position groups per chunk
NITER = NCH * K2  # 72 (c, k) iterations, chunk-major
NGI = NITER * NG  # 288 (i, g) steps
NBLK = NITER // 2  # 36 gather blocks of 1024 samples

PADLO = 65        # leading pad rows in the padded [HW, C] image
R2 = 4292         # padded image rows
R3 = 4232         # patch-array rows (4225 used)
MAGIC = 12582912.0  # 1.5 * 2**23, round-to-nearest-even on f32


def build_nc():
    nc = bacc.Bacc("TRN2", detect_race_conditions=False)

    in3 = nc.dram_tensor("in3", [R3, 1024], BF16, kind="ExternalInput")
    wT = nc.dram_tensor("wT", [2304, 256], BF16, kind="ExternalInput")
    ident16m = nc.dram_tensor("ident16m", [128, 2048], BF16, kind="ExternalInput")
    biasm = nc.dram_tensor("biasm", [128, 2], F32, kind="ExternalInput")
    dyW = nc.dram_tensor("dyW", [128, 288], F32, kind="ExternalInput")
    dxW = nc.dram_tensor("dxW", [128, 288], F32, kind="ExternalInput")
    mW = nc.dram_tensor("mW", [128, 288], F32, kind="ExternalInput")
    byW = nc.dram_tensor("byW", [128, 288], F32, kind="ExternalInput")
    bxW = nc.dram_tensor("bxW", [128, 288], F32, kind="ExternalInput")
    dyI = nc.dram_tensor("dyI", [128, 2304], F32, kind="ExternalInput")
    dxI = nc.dram_tensor("dxI", [128, 2304], F32, kind="ExternalInput")
    byI = nc.dram_tensor("byI", [128, 2304], F32, kind="ExternalInput")
    bxI = nc.dram_tensor("bxI", [128, 2304], F32, kind="ExternalInput")
    outT = nc.dram_tensor("out", [256, 4096], F32, kind="ExternalOutput")

    with ExitStack() as ctx:
        ec = ctx.enter_context

        # sbuf
        gdst = [ec(nc.sbuf_tensor(f"gdst{j}", [128, 8192], BF16)) for j in range(2)]
        w_sb = ec(nc.sbuf_tensor("wsb", [128, 4608], BF16))
        ident16 = ec(nc.sbuf_tensor("ident16", [128, 2048], BF16))
        bias_sb = ec(nc.sbuf_tensor("biassb", [128, 2], F32))
        Dd = [ec(nc.sbuf_tensor(f"Dd{j}", [128, 2048], BF16)) for j in range(2)]
        st_sb = [ec(nc.sbuf_tensor(f"stsb{j}", [128, 256], BF16)) for j in range(2)]
        out_sb = [ec(nc.sbuf_tensor(f"outsb{j}", [128, 1024], F32)) for j in range(2)]

        # W-layout field tiles [128, 288] + scratch
        fW_in = {n: ec(nc.sbuf_tensor(f"f_{n}", [128, 288], F32))
                 for n in ("dyW", "dxW", "mW", "byW", "bxW")}
        wc4b = ec(nc.sbuf_tensor("wc4b", [128, 1152], F32))
        wcor = [ec(nc.sbuf_tensor(f"wc{q}", [128, 288], F32)) for q in range(4)]
        tw = [ec(nc.sbuf_tensor(f"tw{j}", [128, 288], F32)) for j in range(7)]

        # I-layout tiles [128, 2304] for the gather-index chain
        fI_in = {n: ec(nc.sbuf_tensor(f"f_{n}", [128, 2304], F32))
                 for n in ("dyI", "dxI", "byI", "bxI")}
        ti = [ec(nc.sbuf_tensor(f"ti{j}", [128, 2304], F32)) for j in range(4)]
        jw = ec(nc.sbuf_tensor("jw", [128, 2304], I16))

        # psum: full f32 banks
        psT = [ec(nc.psum_tensor(f"psT{j}", [128, 512], F32)) for j in range(2)]
        psE = [ec(nc.psum_tensor(f"psE{g}", [128, 512], F32)) for g in range(NG)]

        sem_ld = ec(nc.semaphore("sem_ld"))
        sem_prep = ec(nc.semaphore("sem_prep"))
        sem_prep2 = ec(nc.semaphore("sem_prep2"))
        sem_gat = ec(nc.semaphore("sem_gat"))
        sem_dve = ec(nc.semaphore("sem_dve"))
        sem_pet = ec(nc.semaphore("sem_pet"))
        sem_act = ec(nc.semaphore("sem_act"))
        sem_pee = ec(nc.semaphore("sem_pee"))
        sem_epi = ec(nc.semaphore("sem_epi"))
        sem_out = ec(nc.semaphore("sem_out"))

        # ---- AP helpers (flat element offsets) ----
        def sb(t, off, free, count=128):
            pstep = t.shape[1] if len(t.shape) == 2 else int(np.prod(t.shape[1:]))
            return AP(t, off, [[pstep, count], [1, free]])

        def scl(t, col):
            return AP(t, col, [[t.shape[1], 128], [1, 1]])

        loads = [
            (sb(ident16, 0, 2048), AP(ident16m, 0, [[2048, 128], [1, 2048]])),
            (sb(bias_sb, 0, 2), AP(biasm, 0, [[2, 128], [1, 2]])),
        ]
        for n, src in (("dyW", dyW), ("dxW", dxW), ("mW", mW), ("byW", byW),
                       ("bxW", bxW)):
            loads.append((sb(fW_in[n], 0, 288), AP(src, 0, [[288, 128], [1, 288]])))
        for n, src in (("dyI", dyI), ("dxI", dxI), ("byI", byI), ("bxI", bxI)):
            loads.append((sb(fI_in[n], 0, 2304), AP(src, 0, [[2304, 128], [1, 2304]])))
        for kcb in range(18):
            loads.append((sb(w_sb, kcb * 256, 256),
                          AP(wT, kcb * 128 * 256, [[256, 128], [1, 256]])))
        n_loads = len(loads)

        glast = {}  # gi of last corner-mm step of each chunk -> chunk
        for c in range(NCH):
            glast[(c * K2 + (K2 - 1)) * NG + (NG - 1)] = c

        with nc.Block() as block:

            @block.sync
            def _(sync):
                for dst, src in loads:
                    sync.dma_start(dst, src).then_inc(sem_ld, 16)
                for c in range(NCH):
                    sync.wait_ge(sem_epi, 8 * (c + 1))
                    for ob in range(2):
                        dst = AP(outT, ob * 128 * 4096 + c * 512,
                                 [[4096, 128], [1, 512]])
                        src = sb(out_sb[c % 2], ob * 512, 512)
                        sync.dma_start(dst, src).then_inc(sem_out, 16)

            @block.gpsimd
            def _(gp):
                gp.load_library(mlp)
                gp.wait_ge(sem_prep, 1)
                in3_ap = AP(in3, 0, [[1024, R3], [1, 1024]])

                def trig(b):
                    # fire gather block b; its dst gdst[b%2] was last read by
                    # the corner-matmuls of iters {2(b-2), 2(b-2)+1}
                    gp.wait_ge(sem_prep2, b + 1)
                    if b >= 2:
                        gp.wait_ge(sem_pet, 8 * b - 8)
                    gp.trigger_dma(count=1)

                for b in range(NBLK):
                    dst = AP(gdst[b % 2], 0, [[8192, 128], [1024, 8], [1, 1024]])
                    idx = AP(jw, b * 64, [[2304, 128], [1, 64]])
                    gp.dma_gather(dst, in3_ap, idx, 1024, 1024, 1024,
                                  prepare_only=True, sem=sem_gat).then_inc(
                        sem_prep2, 1)
                    if b >= 1:
                        trig(b - 1)
                trig(NBLK - 1)

            @block.vector
            def _(v):
                A = mybir.AluOpType
                v.wait_ge(sem_ld, 16 * n_loads)

                class _G:
                    # auto-insert drain on same-engine RAW/WAR/WAW hazards
                    def __init__(self, eng):
                        self.e = eng
                        self.r = set()
                        self.w = set()

                    def drain(self):
                        self.e.drain()
                        self.r.clear()
                        self.w.clear()

                    def _run(self, fn, outs, ins, args, kwargs):
                        on = {o.tensor.name for o in outs}
                        innames = {a.tensor.name for a in ins
                                   if isinstance(a, AP)}
                        if (on & (self.r | self.w)) or (innames & self.w):
                            self.drain()
                        self.r |= innames
                        self.w |= on
                        return fn(*args, **kwargs)

                    def tensor_add(self, o, a, b):
                        return self._run(self.e.tensor_add, [o], [a, b],
                                         (o, a, b), {})

                    def tensor_sub(self, o, a, b):
                        return self._run(self.e.tensor_sub, [o], [a, b],
                                         (o, a, b), {})

                    def tensor_mul(self, o, a, b):
                        return self._run(self.e.tensor_mul, [o], [a, b],
                                         (o, a, b), {})

                    def tensor_tensor(self, o, a, b, op):
                        return self._run(self.e.tensor_tensor, [o], [a, b],
                                         (o, a, b, op), {})

                    def tensor_scalar(self, o, a, s1, s2, op0, op1):
                        return self._run(self.e.tensor_scalar, [o], [a],
                                         (o, a, s1, s2, op0, op1), {})

                    def tensor_single_scalar(self, o, a, s, op):
                        ins = [a] + ([s] if isinstance(s, AP) else [])
                        return self._run(self.e.tensor_single_scalar, [o], ins,
                                         (o, a, s, op), {})

                    def tensor_copy(self, o, a):
                        return self._run(self.e.tensor_copy, [o], [a],
                                         (o, a), {})

                    def scalar_tensor_tensor(self, o, a, s, b, op0, op1):
                        ins = [a, b] + ([s] if isinstance(s, AP) else [])
                        return self._run(self.e.scalar_tensor_tensor, [o], ins,
                                         (o, a, s, b, op0, op1), {})

                vg = _G(v)

                # ---- gather-index chain (I-layout [128, 2304]) ----
                FI = 2304
                s1, s2, s3, s4 = (sb(t, 0, FI) for t in ti)
                vg.tensor_add(s1, sb(fI_in["dyI"], 0, FI), sb(fI_in["byI"], 0, FI))
                vg.tensor_scalar(s2, s1, MAGIC, MAGIC, A.add, A.subtract)
                vg.tensor_tensor(s3, s2, s1, A.is_gt)
                vg.tensor_sub(s1, s2, s3)                       # ey
                vg.tensor_scalar(s2, s1, -1.0, 63.0, A.max, A.min)  # py
                vg.tensor_add(s3, sb(fI_in["dxI"], 0, FI), sb(fI_in["bxI"], 0, FI))
                vg.tensor_scalar(s1, s3, MAGIC, MAGIC, A.add, A.subtract)
                vg.tensor_tensor(s4, s1, s3, A.is_gt)
                vg.tensor_sub(s1, s1, s4)                       # ex
                vg.tensor_scalar(s3, s1, -1.0, 63.0, A.max, A.min)  # px
                vg.tensor_scalar(s1, s3, float(PADLO), 0.0, A.add, A.add)
                vg.scalar_tensor_tensor(s4, s2, 64.0, s1, A.mult, A.add)  # J
                vg.tensor_copy(AP(jw, 0, [[2304, 128], [1, FI]]), s4).then_inc(
                    sem_prep)

                # ---- corner-weight fields (W-layout [128, 288]) ----
                FW = 288
                t1, t2, t3, t4, t5, t6, t7 = (sb(t, 0, FW) for t in tw)
                dy = sb(fW_in["dyW"], 0, FW)
                dx = sb(fW_in["dxW"], 0, FW)
                mm = sb(fW_in["mW"], 0, FW)
                by = sb(fW_in["byW"], 0, FW)
                bx = sb(fW_in["bxW"], 0, FW)

                vg.tensor_add(t1, dy, by)                          # yA
                vg.tensor_scalar(t2, t1, MAGIC, MAGIC, A.add, A.subtract)
                vg.tensor_tensor(t3, t2, t1, A.is_gt)
                vg.tensor_sub(t4, t2, t3)                          # ey
                vg.tensor_sub(t5, t1, t4)                          # ly
                vg.tensor_scalar(t1, t5, -1.0, 1.0, A.mult, A.add)  # hy
                vg.tensor_single_scalar(t2, t4, 0.0, A.is_ge)
                vg.tensor_single_scalar(t3, t4, 63.0, A.is_le)
                vg.tensor_mul(t2, t2, t3)                          # vy0
                vg.tensor_single_scalar(t3, t4, -1.0, A.is_ge)
                vg.tensor_single_scalar(t6, t4, 62.0, A.is_le)
                vg.tensor_mul(t3, t3, t6)                          # vy1
                vg.tensor_mul(t2, t2, t1)
                vg.tensor_mul(t3, t3, t5)
                vg.tensor_mul(t2, t2, mm)                          # wy0m
                vg.tensor_mul(t3, t3, mm)                          # wy1m

                vg.tensor_add(t1, dx, bx)                          # xA
                vg.tensor_scalar(t4, t1, MAGIC, MAGIC, A.add, A.subtract)
                vg.tensor_tensor(t5, t4, t1, A.is_gt)
                vg.tensor_sub(t4, t4, t5)                          # ex
                vg.tensor_sub(t5, t1, t4)                          # lx
                vg.tensor_scalar(t1, t5, -1.0, 1.0, A.mult, A.add)  # hx
                vg.tensor_single_scalar(t6, t4, 0.0, A.is_ge)
                vg.tensor_single_scalar(t7, t4, 63.0, A.is_le)
                vg.tensor_mul(t6, t6, t7)                          # vx0
                vg.tensor_mul(t6, t6, t1)                          # cx0 = hx*vx0
                vg.tensor_single_scalar(t7, t4, -1.0, A.is_ge)
                vg.tensor_single_scalar(t4, t4, 62.0, A.is_le)
                vg.tensor_mul(t7, t7, t4)                          # vx1
                vg.tensor_mul(t7, t7, t5)                          # cx1 = lx*vx1

                vg.tensor_mul(sb(wcor[0], 0, FW), t2, t6)          # w00
                vg.tensor_mul(sb(wcor[1], 0, FW), t2, t7)          # w01
                vg.tensor_mul(sb(wcor[2], 0, FW), t3, t6)          # w10
                vg.tensor_mul(sb(wcor[3], 0, FW), t3, t7)          # w11

                # relayout wcor[q][p, k*32+c*4+g] -> wc4b[p, (c*9+k)*16+q*4+g]
                for q in range(4):
                    vg.tensor_copy(
                        AP(wc4b, q * 4, [[1152, 128], [144, 8], [16, 9], [1, 4]]),
                        AP(wcor[q], 0, [[288, 128], [4, 8], [32, 9], [1, 4]]))

                # ---- per-iteration diag build ----
                # Dd[i%2][p, (q*4+g)*128 + j] = ident16[p, j] * w_{q,g}[p...]
                iden_ap = AP(ident16, 0, [[2048, 128], [0, 16], [1, 128]])
                for i in range(NITER):
                    c, k = divmod(i, K2)
                    if i >= 2:
                        v.wait_ge(sem_pet, NG * (i - 1))
                    wsrc = AP(wc4b, i * 16, [[1152, 128], [1, 16], [0, 128]])
                    vg.tensor_tensor(
                        AP(Dd[i % 2], 0, [[2048, 128], [1, 2048]]),
                        iden_ap, wsrc, A.mult).then_inc(sem_dve)

            @block.tensor
            def _(te):
                te.wait_ge(sem_ld, 16 * n_loads)

                def emit_einsum(gg):
                    i2, g2 = divmod(gg, NG)
                    c2, k2 = divmod(i2, K2)
                    te.wait_ge(sem_act, gg + 1)
                    if k2 == 0 and c2 >= 1:
                        # psE[g2] bank reused across chunks; wait for the
                        # previous chunk's epilogue to finish reading it
                        te.wait_ge(sem_epi, 8 * c2)
                    last = None
                    for ob in range(2):
                        for cb in range(2):
                            lhs = AP(w_sb, (k2 * 2 + cb) * 256 + ob * 128,
                                     [[4608, 128], [1, 128]])
                            rhs = AP(st_sb[gg % 2], cb * 128, [[256, 128], [1, 128]])
                            dst = AP(psE[g2], ob * 128, [[512, 128], [1, 128]])
                            last = te.matmul(dst, lhs, rhs,
                                             start=(k2 == 0 and ob == 0
                                                    and cb == 0),
                                             stop=(k2 == K2 - 1 and ob == 1
                                                   and cb == 1))
                    last.then_inc(sem_pee)

                for i in range(NITER):
                    blk = i // 2
                    for g in range(NG):
                        gi = NG * i + g
                        te.wait_ge(sem_gat, 16 * (blk + 1))
                        te.wait_ge(sem_dve, i + 1)
                        if gi >= 2:
                            te.wait_ge(sem_act, gi - 1)
                        last = None
                        for q in range(4):
                            for cb in range(2):
                                lhs = AP(gdst[blk % 2],
                                         (i % 2) * 4096 + g * 1024 + q * 256
                                         + cb * 128,
                                         [[8192, 128], [1, 128]])
                                rhs = AP(Dd[i % 2], (q * 4 + g) * 128,
                                         [[2048, 128], [1, 128]])
                                dst = AP(psT[gi % 2], cb * 128,
                                         [[512, 128], [1, 128]])
                                last = te.matmul(dst, lhs, rhs,
                                                 start=(q == 0 and cb == 0),
                                                 stop=(q == 3 and cb == 1))
                        last.then_inc(sem_pet)
                        if gi >= 1:
                            emit_einsum(gi - 1)
                emit_einsum(NGI - 1)

            @block.scalar
            def _(sc):
                IDENT = mybir.ActivationFunctionType.Identity
                for gi in range(NGI):
                    sc.wait_ge(sem_pet, gi + 1)
                    if gi >= 2:
                        sc.wait_ge(sem_pee, gi - 1)
                    sc.activation(sb(st_sb[gi % 2], 0, 256),
                                  AP(psT[gi % 2], 0, [[512, 128], [1, 256]]),
                                  IDENT).then_inc(sem_act)
                    c = glast.get(gi)
                    if c is not None:
                        if c >= 2:
                            sc.wait_ge(sem_out, 32 * (c - 1))
                        for g2 in range(NG):
                            sc.wait_ge(sem_pee, (c * K2 + K2 - 1) * NG + g2 + 1)
                            for ob in range(2):
                                sc.activation(
                                    sb(out_sb[c % 2], ob * 512 + g2 * 128, 128),
                                    AP(psE[g2], ob * 128,
                                       [[512, 128], [1, 128]]),
                                    IDENT, bias=scl(bias_sb, ob),
                                ).then_inc(sem_epi)

    nc.compile()
    return nc


# ---------------------------------------------------------------------------
# host marshalling
# ---------------------------------------------------------------------------

def _to_W(f):
    # f [9, 4096] -> [128, 288]; fW[p%128, k*32 + p//128] = f[k, p]
    return np.ascontiguousarray(
        f.reshape(9, 32, 128).transpose(2, 0, 1).reshape(128, 288))


def _to_I(f):
    # f [9, 4096] -> wrapped [128, 2304]; fI[r, (c*9+k)*32+t] = f[k, c*512+t*16+r%16]
    # (c,k)-major so gather block b covers samples [1024b, 1024b+1024)
    a = f.reshape(9, 8, 32, 16).transpose(3, 1, 0, 2).reshape(16, 2304)
    return np.ascontiguousarray(np.tile(a, (8, 1)))


def marshal(inputs):
    import ml_dtypes
    np_bf16 = ml_dtypes.bfloat16

    inp = np.asarray(inputs["input"], np.float32)
    off = np.asarray(inputs["offset"], np.float32)
    msk = np.asarray(inputs["mask"], np.float32)
    wgt = np.asarray(inputs["weight"], np.float32)
    bias = np.asarray(inputs["bias"], np.float32)

    wT = np.ascontiguousarray(
        wgt.reshape(O, C, K2).transpose(2, 1, 0).reshape(2304, 256)).astype(np_bf16)
    biasm = np.ascontiguousarray(bias.reshape(2, 128).T)
    ident16 = np.zeros((128, 2048), np_bf16)
    for p in range(128):
        ident16[p, p::128] = 1.0

    ho = np.arange(HW, dtype=np.float32) // 64
    wo = np.arange(HW, dtype=np.float32) % 64
    ks = np.arange(K2, dtype=np.float32)
    by = ho[None, :] - 1.0 + (ks // 3)[:, None]
    bx = wo[None, :] - 1.0 + (ks % 3)[:, None]

    shared = {
        "wT": wT, "ident16m": ident16, "biasm": biasm,
        "byW": _to_W(by), "bxW": _to_W(bx),
        "byI": _to_I(by), "bxI": _to_I(bx),
    }

    in_maps = []
    for b in range(B):
        img = inp[b].transpose(1, 2, 0).reshape(HW, C)
        in2p = np.zeros((R2, C), np.float32)
        in2p[PADLO:PADLO + HW] = img
        in3 = np.zeros((R3, 1024), np.float32)
        n = HW + 2 * PADLO - 1  # 4225 usable rows
        in3[:n, 0:256] = in2p[0:n]
        in3[:n, 256:512] = in2p[1:n + 1]
        in3[:n, 512:768] = in2p[64:n + 64]
        in3[:n, 768:1024] = in2p[65:n + 65]

        off_y = np.ascontiguousarray(off[b, 0::2].reshape(K2, HW))
        off_x = np.ascontiguousarray(off[b, 1::2].reshape(K2, HW))
        im = {
            "in3": in3.astype(np_bf16),
            "dyW": _to_W(off_y), "dxW": _to_W(off_x),
            "mW": _to_W(msk[b].reshape(K2, HW)),
            "dyI": _to_I(off_y), "dxI": _to_I(off_x),
        }
        im.update(shared)
        in_maps.append(im)
    return in_maps


_NC_CACHE = {}


def _get_nc():
    if "nc" not in _NC_CACHE:
        _NC_CACHE["nc"] = build_nc()
    return _NC_CACHE["nc"]


def run(inputs, trace=False, **kw):
    nc = _get_nc()
    in_maps = marshal(inputs)
    res = bass_utils.run_bass_kernel_spmd(nc, in_maps, core_ids=list(range(B)),
                                          trace=trace, **kw)
    out = np.stack([r["out"].reshape(O, H, W) for r in res.results])
    return out.astype(np.float32), res


def kernel(**inputs):
    return run(inputs)[0]


# revision 7
# speedup vs baseline: 2.3048x; 1.2446x over previous
"""Modulated deformable conv v2 (B=8, C=O=256, H=W=64, 3x3) on 8 trn2 NeuronCores.

Strategy: data-parallel over batch (1 image per core). Per core:
  - host marshals the image into a "patch array" in3[j] = concat of padded
    [HW, C] rows (j, j+1, j+64, j+65) in bf16 so one contiguous 2KB DMA
    descriptor fetches the full 2x2 bilinear patch for all 256 channels of
    one sample.
  - device computes patch indices (floor/clip of offsets) on DVE, gpsimd
    dma_gather streams patch rows (HBM->SBUF) in 1024-sample blocks.
  - corner weighting + transpose + corner-sum are fused on the PE: for each
    128-position group, 8 matmuls  psT[ch, pos] += gdst_q[pos, ch]^T @
    diag(w_q[pos])  accumulate the 4 bilinear corners (weights fold
    bilinear frac * validity * modulation mask).  The diagonal tiles are
    rebuilt per iteration by one DVE op: Dd = ident16 * broadcast(w).
  - ACT copies PSUM->SBUF (cast bf16), PE runs the O x (C*K2) x HW einsum
    with PSUM accumulation, ACT folds the bias, HWDGE DMAs the output back.
"""

import numpy as np
from contextlib import ExitStack

import concourse.bacc as bacc
import concourse.bass as bass
import concourse.mybir as mybir
from concourse import bass_utils
from concourse.library_config import mlp

AP = bass.AP
F32 = mybir.dt.float32
BF16 = mybir.dt.bfloat16
I16 = mybir.dt.int16

# problem constants (hardcoded per contract)
B = 8
C = 256
O = 256
H = W = 64
HW = 4096
K2 = 9

NCH = 8           # spatial chunks
NG = 4            # 128-position groups per chunk
NITER = NCH * K2  # 72 (c, k) iterations, chunk-major
NGI = NITER * NG  # 288 (i, g) steps
NBLK = NITER // 2  # 36 gather blocks of 1024 samples

PADLO = 65        # leading pad rows in the padded [HW, C] image
R2 = 4292         # padded image rows
R3 = 4232         # patch-array rows (4225 used)
MAGIC = 12582912.0  # 1.5 * 2**23, round-to-nearest-even on f32


def build_nc():
    nc = bacc.Bacc("TRN2", detect_race_conditions=False,
                   dynamic_dma_scratch_size=32768, num_swdge_queues=2)

    in3 = nc.dram_tensor("in3", [R3, 1024], BF16, kind="ExternalInput")
    wT = nc.dram_tensor("wT", [2304, 256], BF16, kind="ExternalInput")
    ident16m = nc.dram_tensor("ident16m", [128, 2048], BF16, kind="ExternalInput")
    biasm = nc.dram_tensor("biasm", [128, 2], F32, kind="ExternalInput")
    dyW = nc.dram_tensor("dyW", [128, 288], F32, kind="ExternalInput")
    dxW = nc.dram_tensor("dxW", [128, 288], F32, kind="ExternalInput")
    mW = nc.dram_tensor("mW", [128, 288], F32, kind="ExternalInput")
    byW = nc.dram_tensor("byW", [128, 288], F32, kind="ExternalInput")
    bxW = nc.dram_tensor("bxW", [128, 288], F32, kind="ExternalInput")
    dyI = nc.dram_tensor("dyI", [128, 2304], F32, kind="ExternalInput")
    dxI = nc.dram_tensor("dxI", [128, 2304], F32, kind="ExternalInput")
    byI = nc.dram_tensor("byI", [128, 2304], F32, kind="ExternalInput")
    bxI = nc.dram_tensor("bxI", [128, 2304], F32, kind="ExternalInput")
    outT = nc.dram_tensor("out", [256, 4096], F32, kind="ExternalOutput")

    with ExitStack() as ctx:
        ec = ctx.enter_context

        # sbuf
        gdst = [ec(nc.sbuf_tensor(f"gdst{j}", [128, 8192], BF16)) for j in range(2)]
        w_sb = ec(nc.sbuf_tensor("wsb", [128, 4608], BF16))
        ident16 = ec(nc.sbuf_tensor("ident16", [128, 2048], BF16))
        bias_sb = ec(nc.sbuf_tensor("biassb", [128, 2], F32))
        Dd = [ec(nc.sbuf_tensor(f"Dd{j}", [128, 2048], BF16)) for j in range(2)]
        st_sb = [ec(nc.sbuf_tensor(f"stsb{j}", [128, 256], BF16)) for j in range(2)]
        out_sb = [ec(nc.sbuf_tensor(f"outsb{j}", [128, 1024], F32)) for j in range(2)]

        # W-layout field tiles [128, 288] + scratch
        fW_in = {n: ec(nc.sbuf_tensor(f"f_{n}", [128, 288], F32))
                 for n in ("dyW", "dxW", "mW", "byW", "bxW")}
        wc4b = ec(nc.sbuf_tensor("wc4b", [128, 1152], F32))
        wcor = [ec(nc.sbuf_tensor(f"wc{q}", [128, 288], F32)) for q in range(4)]
        tw = [ec(nc.sbuf_tensor(f"tw{j}", [128, 288], F32)) for j in range(7)]

        # I-layout tiles [128, 2304] for the gather-index chain
        fI_in = {n: ec(nc.sbuf_tensor(f"f_{n}", [128, 2304], F32))
                 for n in ("dyI", "dxI", "byI", "bxI")}
        ti = [ec(nc.sbuf_tensor(f"ti{j}", [128, 2304], F32)) for j in range(4)]
        jw = ec(nc.sbuf_tensor("jw", [128, 2304], I16))

        # psum: full f32 banks
        psT = [ec(nc.psum_tensor(f"psT{j}", [128, 512], F32)) for j in range(2)]
        psE = [ec(nc.psum_tensor(f"psE{g}", [128, 512], F32)) for g in range(NG)]

        sem_ld = ec(nc.semaphore("sem_ld"))
        sem_prep = ec(nc.semaphore("sem_prep"))
        sem_prep2 = ec(nc.semaphore("sem_prep2"))
        sem_gat = ec(nc.semaphore("sem_gat"))
        sem_dve = ec(nc.semaphore("sem_dve"))
        sem_pet = ec(nc.semaphore("sem_pet"))
        sem_act = ec(nc.semaphore("sem_act"))
        sem_pee = ec(nc.semaphore("sem_pee"))
        sem_epi = ec(nc.semaphore("sem_epi"))
        sem_out = ec(nc.semaphore("sem_out"))

        # ---- AP helpers (flat element offsets) ----
        def sb(t, off, free, count=128):
            pstep = t.shape[1] if len(t.shape) == 2 else int(np.prod(t.shape[1:]))
            return AP(t, off, [[pstep, count], [1, free]])

        def scl(t, col):
            return AP(t, col, [[t.shape[1], 128], [1, 1]])

        loads = [
            (sb(ident16, 0, 2048), AP(ident16m, 0, [[2048, 128], [1, 2048]])),
            (sb(bias_sb, 0, 2), AP(biasm, 0, [[2, 128], [1, 2]])),
        ]
        for n, src in (("dyW", dyW), ("dxW", dxW), ("mW", mW), ("byW", byW),
                       ("bxW", bxW)):
            loads.append((sb(fW_in[n], 0, 288), AP(src, 0, [[288, 128], [1, 288]])))
        for n, src in (("dyI", dyI), ("dxI", dxI), ("byI", byI), ("bxI", bxI)):
            loads.append((sb(fI_in[n], 0, 2304), AP(src, 0, [[2304, 128], [1, 2304]])))
        for kcb in range(18):
            loads.append((sb(w_sb, kcb * 256, 256),
                          AP(wT, kcb * 128 * 256, [[256, 128], [1, 256]])))
        n_loads = len(loads)

        glast = {}  # gi of last corner-mm step of each chunk -> chunk
        for c in range(NCH):
            glast[(c * K2 + (K2 - 1)) * NG + (NG - 1)] = c

        with nc.Block() as block:

            @block.sync
            def _(sync):
                for dst, src in loads:
                    sync.dma_start(dst, src).then_inc(sem_ld, 16)
                for c in range(NCH):
                    sync.wait_ge(sem_epi, 8 * (c + 1))
                    for ob in range(2):
                        dst = AP(outT, ob * 128 * 4096 + c * 512,
                                 [[4096, 128], [1, 512]])
                        src = sb(out_sb[c % 2], ob * 512, 512)
                        sync.dma_start(dst, src).then_inc(sem_out, 16)

            @block.gpsimd
            def _(gp):
                gp.load_library(mlp)
                in3_ap = AP(in3, 0, [[1024, R3], [1, 1024]])

                def trig(b):
                    # fire gather block b; its dst gdst[b%2] was last read by
                    # the corner-matmuls of iters {2(b-2), 2(b-2)+1}
                    gp.wait_ge(sem_prep2, b + 1)
                    if b >= 2:
                        gp.wait_ge(sem_pet, 8 * b - 8)
                    gp.trigger_dma(count=1, queue_num=b % 2)

                for b in range(NBLK):
                    # jw ready: first 4 blocks after sem_prep=1, rest after 2
                    gp.wait_ge(sem_prep, 1 if b < 4 else 2)
                    dst = AP(gdst[b % 2], 0, [[8192, 128], [1024, 8], [1, 1024]])
                    idx = AP(jw, b * 64, [[2304, 128], [1, 64]])
                    gp.dma_gather(dst, in3_ap, idx, 1024, 1024, 1024,
                                  prepare_only=True, sem=sem_gat,
                                  queue_num=b % 2).then_inc(sem_prep2, 1)
                    if b >= 1:
                        trig(b - 1)
                trig(NBLK - 1)

            @block.vector
            def _(v):
                A = mybir.AluOpType
                v.wait_ge(sem_ld, 16 * n_loads)

                class _G:
                    # auto-insert drain on same-engine RAW/WAR/WAW hazards
                    def __init__(self, eng):
                        self.e = eng
                        self.r = set()
                        self.w = set()

                    def drain(self):
                        self.e.drain()
                        self.r.clear()
                        self.w.clear()

                    def _run(self, fn, outs, ins, args, kwargs):
                        on = {o.tensor.name for o in outs}
                        innames = {a.tensor.name for a in ins
                                   if isinstance(a, AP)}
                        if (on & (self.r | self.w)) or (innames & self.w):
                            self.drain()
                        self.r |= innames
                        self.w |= on
                        return fn(*args, **kwargs)

                    def tensor_add(self, o, a, b):
                        return self._run(self.e.tensor_add, [o], [a, b],
                                         (o, a, b), {})

                    def tensor_sub(self, o, a, b):
                        return self._run(self.e.tensor_sub, [o], [a, b],
                                         (o, a, b), {})

                    def tensor_mul(self, o, a, b):
                        return self._run(self.e.tensor_mul, [o], [a, b],
                                         (o, a, b), {})

                    def tensor_tensor(self, o, a, b, op):
                        return self._run(self.e.tensor_tensor, [o], [a, b],
                                         (o, a, b, op), {})

                    def tensor_scalar(self, o, a, s1, s2, op0, op1):
                        return self._run(self.e.tensor_scalar, [o], [a],
                                         (o, a, s1, s2, op0, op1), {})

                    def tensor_single_scalar(self, o, a, s, op):
                        ins = [a] + ([s] if isinstance(s, AP) else [])
                        return self._run(self.e.tensor_single_scalar, [o], ins,
                                         (o, a, s, op), {})

                    def tensor_copy(self, o, a):
                        return self._run(self.e.tensor_copy, [o], [a],
                                         (o, a), {})

                    def scalar_tensor_tensor(self, o, a, s, b, op0, op1):
                        ins = [a, b] + ([s] if isinstance(s, AP) else [])
                        return self._run(self.e.scalar_tensor_tensor, [o], ins,
                                         (o, a, s, b, op0, op1), {})

                vg = _G(v)

                # ---- gather-index chain (I-layout [128, 2304]) ----
                # two passes: first 4 blocks' columns for a fast pipeline
                # start, then the rest
                def jchain(off, width):
                    s1, s2, s3, s4 = (sb(t, off, width) for t in ti)
                    vg.tensor_add(s1, sb(fI_in["dyI"], off, width),
                                  sb(fI_in["byI"], off, width))
                    vg.tensor_scalar(s2, s1, MAGIC, MAGIC, A.add, A.subtract)
                    vg.tensor_tensor(s3, s2, s1, A.is_gt)
                    vg.tensor_sub(s1, s2, s3)                       # ey
                    vg.tensor_scalar(s2, s1, -1.0, 63.0, A.max, A.min)  # py
                    vg.tensor_add(s3, sb(fI_in["dxI"], off, width),
                                  sb(fI_in["bxI"], off, width))
                    vg.tensor_scalar(s1, s3, MAGIC, MAGIC, A.add, A.subtract)
                    vg.tensor_tensor(s4, s1, s3, A.is_gt)
                    vg.tensor_sub(s1, s1, s4)                       # ex
                    vg.tensor_scalar(s3, s1, -1.0, 63.0, A.max, A.min)  # px
                    vg.tensor_scalar(s1, s3, float(PADLO), 0.0, A.add, A.add)
                    vg.scalar_tensor_tensor(s4, s2, 64.0, s1, A.mult, A.add)
                    vg.tensor_copy(AP(jw, off, [[2304, 128], [1, width]]),
                                   s4).then_inc(sem_prep)

                jchain(0, 256)
                jchain(256, 2048)

                # ---- corner-weight fields (W-layout [128, 288]) ----
                FW = 288
                t1, t2, t3, t4, t5, t6, t7 = (sb(t, 0, FW) for t in tw)
                dy = sb(fW_in["dyW"], 0, FW)
                dx = sb(fW_in["dxW"], 0, FW)
                mm = sb(fW_in["mW"], 0, FW)
                by = sb(fW_in["byW"], 0, FW)
                bx = sb(fW_in["bxW"], 0, FW)

                vg.tensor_add(t1, dy, by)                          # yA
                vg.tensor_scalar(t2, t1, MAGIC, MAGIC, A.add, A.subtract)
                vg.tensor_tensor(t3, t2, t1, A.is_gt)
                vg.tensor_sub(t4, t2, t3)                          # ey
                vg.tensor_sub(t5, t1, t4)                          # ly
                vg.tensor_scalar(t1, t5, -1.0, 1.0, A.mult, A.add)  # hy
                vg.tensor_single_scalar(t2, t4, 0.0, A.is_ge)
                vg.tensor_single_scalar(t3, t4, 63.0, A.is_le)
                vg.tensor_mul(t2, t2, t3)                          # vy0
                vg.tensor_single_scalar(t3, t4, -1.0, A.is_ge)
                vg.tensor_single_scalar(t6, t4, 62.0, A.is_le)
                vg.tensor_mul(t3, t3, t6)                          # vy1
                vg.tensor_mul(t2, t2, t1)
                vg.tensor_mul(t3, t3, t5)
                vg.tensor_mul(t2, t2, mm)                          # wy0m
                vg.tensor_mul(t3, t3, mm)                          # wy1m

                vg.tensor_add(t1, dx, bx)                          # xA
                vg.tensor_scalar(t4, t1, MAGIC, MAGIC, A.add, A.subtract)
                vg.tensor_tensor(t5, t4, t1, A.is_gt)
                vg.tensor_sub(t4, t4, t5)                          # ex
                vg.tensor_sub(t5, t1, t4)                          # lx
                vg.tensor_scalar(t1, t5, -1.0, 1.0, A.mult, A.add)  # hx
                vg.tensor_single_scalar(t6, t4, 0.0, A.is_ge)
                vg.tensor_single_scalar(t7, t4, 63.0, A.is_le)
                vg.tensor_mul(t6, t6, t7)                          # vx0
                vg.tensor_mul(t6, t6, t1)                          # cx0 = hx*vx0
                vg.tensor_single_scalar(t7, t4, -1.0, A.is_ge)
                vg.tensor_single_scalar(t4, t4, 62.0, A.is_le)
                vg.tensor_mul(t7, t7, t4)                          # vx1
                vg.tensor_mul(t7, t7, t5)                          # cx1 = lx*vx1

                vg.tensor_mul(sb(wcor[0], 0, FW), t2, t6)          # w00
                vg.tensor_mul(sb(wcor[1], 0, FW), t2, t7)          # w01
                vg.tensor_mul(sb(wcor[2], 0, FW), t3, t6)          # w10
                vg.tensor_mul(sb(wcor[3], 0, FW), t3, t7)          # w11

                # relayout wcor[q][p, k*32+c*4+g] -> wc4b[p, (c*9+k)*16+q*4+g]
                for q in range(4):
                    vg.tensor_copy(
                        AP(wc4b, q * 4, [[1152, 128], [144, 8], [16, 9], [1, 4]]),
                        AP(wcor[q], 0, [[288, 128], [4, 8], [32, 9], [1, 4]]))

                # ---- per-iteration diag build ----
                # Dd[i%2][p, (q*4+g)*128 + j] = ident16[p, j] * w_{q,g}[p...]
                iden_ap = AP(ident16, 0, [[2048, 128], [0, 16], [1, 128]])
                for i in range(NITER):
                    c, k = divmod(i, K2)
                    if i >= 2:
                        v.wait_ge(sem_pet, NG * (i - 1))
                    wsrc = AP(wc4b, i * 16, [[1152, 128], [1, 16], [0, 128]])
                    vg.tensor_tensor(
                        AP(Dd[i % 2], 0, [[2048, 128], [1, 2048]]),
                        iden_ap, wsrc, A.mult).then_inc(sem_dve)

            @block.tensor
            def _(te):
                te.wait_ge(sem_ld, 16 * n_loads)

                def emit_einsum(gg):
                    i2, g2 = divmod(gg, NG)
                    c2, k2 = divmod(i2, K2)
                    te.wait_ge(sem_act, gg + 1)
                    if k2 == 0 and c2 >= 1:
                        # psE[g2] bank reused across chunks; wait for the
                        # previous chunk's epilogue to finish reading it
                        te.wait_ge(sem_epi, 8 * c2)
                    last = None
                    for ob in range(2):
                        for cb in range(2):
                            lhs = AP(w_sb, (k2 * 2 + cb) * 256 + ob * 128,
                                     [[4608, 128], [1, 128]])
                            rhs = AP(st_sb[gg % 2], cb * 128, [[256, 128], [1, 128]])
                            dst = AP(psE[g2], ob * 128, [[512, 128], [1, 128]])
                            last = te.matmul(dst, lhs, rhs,
                                             start=(k2 == 0 and ob == 0
                                                    and cb == 0),
                                             stop=(k2 == K2 - 1 and ob == 1
                                                   and cb == 1))
                    last.then_inc(sem_pee)

                for i in range(NITER):
                    blk = i // 2
                    for g in range(NG):
                        gi = NG * i + g
                        te.wait_ge(sem_gat, 16 * (blk + 1))
                        te.wait_ge(sem_dve, i + 1)
                        if gi >= 2:
                            te.wait_ge(sem_act, gi - 1)
                        last = None
                        for q in range(4):
                            for cb in range(2):
                                lhs = AP(gdst[blk % 2],
                                         (i % 2) * 4096 + g * 1024 + q * 256
                                         + cb * 128,
                                         [[8192, 128], [1, 128]])
                                rhs = AP(Dd[i % 2], (q * 4 + g) * 128,
                                         [[2048, 128], [1, 128]])
                                dst = AP(psT[gi % 2], cb * 128,
                                         [[512, 128], [1, 128]])
                                last = te.matmul(dst, lhs, rhs,
                                                 start=(q == 0 and cb == 0),
                                                 stop=(q == 3 and cb == 1))
                        last.then_inc(sem_pet)
                        if gi >= 1:
                            emit_einsum(gi - 1)
                emit_einsum(NGI - 1)

            @block.scalar
            def _(sc):
                IDENT = mybir.ActivationFunctionType.Identity
                for gi in range(NGI):
                    sc.wait_ge(sem_pet, gi + 1)
                    if gi >= 2:
                        sc.wait_ge(sem_pee, gi - 1)
                    sc.activation(sb(st_sb[gi % 2], 0, 256),
                                  AP(psT[gi % 2], 0, [[512, 128], [1, 256]]),
                                  IDENT).then_inc(sem_act)
                    c = glast.get(gi)
                    if c is not None:
                        if c >= 2:
                            sc.wait_ge(sem_out, 32 * (c - 1))
                        for g2 in range(NG):
                            sc.wait_ge(sem_pee, (c * K2 + K2 - 1) * NG + g2 + 1)
                            for ob in range(2):
                                sc.activation(
                                    sb(out_sb[c % 2], ob * 512 + g2 * 128, 128),
                                    AP(psE[g2], ob * 128,
                                       [[512, 128], [1, 128]]),
                                    IDENT, bias=scl(bias_sb, ob),
                                ).then_inc(sem_epi)

    nc.compile()
    return nc


# ---------------------------------------------------------------------------
# host marshalling
# ---------------------------------------------------------------------------

def _to_W(f):
    # f [9, 4096] -> [128, 288]; fW[p%128, k*32 + p//128] = f[k, p]
    return np.ascontiguousarray(
        f.reshape(9, 32, 128).transpose(2, 0, 1).reshape(128, 288))


def _to_I(f):
    # f [9, 4096] -> wrapped [128, 2304]; fI[r, (c*9+k)*32+t] = f[k, c*512+t*16+r%16]
    # (c,k)-major so gather block b covers samples [1024b, 1024b+1024)
    a = f.reshape(9, 8, 32, 16).transpose(3, 1, 0, 2).reshape(16, 2304)
    return np.ascontiguousarray(np.tile(a, (8, 1)))


def marshal(inputs):
    import ml_dtypes
    np_bf16 = ml_dtypes.bfloat16

    inp = np.asarray(inputs["input"], np.float32)
    off = np.asarray(inputs["offset"], np.float32)
    msk = np.asarray(inputs["mask"], np.float32)
    wgt = np.asarray(inputs["weight"], np.float32)
    bias = np.asarray(inputs["bias"], np.float32)

    wT = np.ascontiguousarray(
        wgt.reshape(O, C, K2).transpose(2, 1, 0).reshape(2304, 256)).astype(np_bf16)
    biasm = np.ascontiguousarray(bias.reshape(2, 128).T)
    ident16 = np.zeros((128, 2048), np_bf16)
    for p in range(128):
        ident16[p, p::128] = 1.0

    ho = np.arange(HW, dtype=np.float32) // 64
    wo = np.arange(HW, dtype=np.float32) % 64
    ks = np.arange(K2, dtype=np.float32)
    by = ho[None, :] - 1.0 + (ks // 3)[:, None]
    bx = wo[None, :] - 1.0 + (ks % 3)[:, None]

    shared = {
        "wT": wT, "ident16m": ident16, "biasm": biasm,
        "byW": _to_W(by), "bxW": _to_W(bx),
        "byI": _to_I(by), "bxI": _to_I(bx),
    }

    in_maps = []
    for b in range(B):
        img = inp[b].transpose(1, 2, 0).reshape(HW, C)
        in2p = np.zeros((R2, C), np.float32)
        in2p[PADLO:PADLO + HW] = img
        in3 = np.zeros((R3, 1024), np.float32)
        n = HW + 2 * PADLO - 1  # 4225 usable rows
        in3[:n, 0:256] = in2p[0:n]
        in3[:n, 256:512] = in2p[1:n + 1]
        in3[:n, 512:768] = in2p[64:n + 64]
        in3[:n, 768:1024] = in2p[65:n + 65]

        off_y = np.ascontiguousarray(off[b, 0::2].reshape(K2, HW))
        off_x = np.ascontiguousarray(off[b, 1::2].reshape(K2, HW))
        im = {
            "in3": in3.astype(np_bf16),
            "dyW": _to_W(off_y), "dxW": _to_W(off_x),
            "mW": _to_W(msk[b].reshape(K2, HW)),
            "dyI": _to_I(off_y), "dxI": _to_I(off_x),
        }
        im.update(shared)
        in_maps.append(im)
    return in_maps


_NC_CACHE = {}


def _get_nc():
    if "nc" not in _NC_CACHE:
        _NC_CACHE["nc"] = build_nc()
    return _NC_CACHE["nc"]


def run(inputs, trace=False, **kw):
    nc = _get_nc()
    in_maps = marshal(inputs)
    res = bass_utils.run_bass_kernel_spmd(nc, in_maps, core_ids=list(range(B)),
                                          trace=trace, **kw)
    out = np.stack([r["out"].reshape(O, H, W) for r in res.results])
    return out.astype(np.float32), res


def kernel(**inputs):
    return run(inputs)[0]


# revision 20
# speedup vs baseline: 2.3888x; 1.0364x over previous
"""Modulated deformable conv v2 (B=8, C=O=256, H=W=64, 3x3) on 8 trn2 NeuronCores.

Strategy: data-parallel over batch (1 image per core). Per core:
  - host marshals the image into a "patch array" in3[j] = concat of padded
    [HW, C] rows (j, j+1, j+64, j+65) in bf16 so one contiguous 2KB DMA
    descriptor fetches the full 2x2 bilinear patch for all 256 channels of
    one sample.
  - device computes patch indices (floor/clip of offsets) on DVE, gpsimd
    dma_gather streams patch rows (HBM->SBUF) in 1024-sample blocks.
  - corner weighting + transpose + corner-sum are fused on the PE: for each
    128-position group, 8 matmuls  psT[ch, pos] += gdst_q[pos, ch]^T @
    diag(w_q[pos])  accumulate the 4 bilinear corners (weights fold
    bilinear frac * validity * modulation mask).  The diagonal tiles are
    rebuilt per iteration by one DVE op: Dd = ident16 * broadcast(w).
  - ACT copies PSUM->SBUF (cast bf16), PE runs the O x (C*K2) x HW einsum
    with PSUM accumulation, ACT folds the bias, HWDGE DMAs the output back.
"""

import numpy as np
from contextlib import ExitStack

import concourse.bacc as bacc
import concourse.bass as bass
import concourse.mybir as mybir
from concourse import bass_utils
from concourse.library_config import mlp

AP = bass.AP
F32 = mybir.dt.float32
BF16 = mybir.dt.bfloat16
I16 = mybir.dt.int16

# problem constants (hardcoded per contract)
B = 8
C = 256
O = 256
H = W = 64
HW = 4096
K2 = 9

NCH = 8           # spatial chunks
NG = 4            # 128-position groups per chunk
NITER = NCH * K2  # 72 (c, k) iterations, chunk-major
NGI = NITER * NG  # 288 (i, g) steps
NBLK = NITER // 2  # 36 gather blocks of 1024 samples

PADLO = 65        # leading pad rows in the padded [HW, C] image
R2 = 4292         # padded image rows
R3 = 4232         # patch-array rows (4225 used)
MAGIC = 12582912.0  # 1.5 * 2**23, round-to-nearest-even on f32


def build_nc():
    nc = bacc.Bacc("TRN2", detect_race_conditions=False,
                   dynamic_dma_scratch_size=32768, num_swdge_queues=2)

    in3 = nc.dram_tensor("in3", [R3, 1024], BF16, kind="ExternalInput")
    wT = nc.dram_tensor("wT", [2304, 256], BF16, kind="ExternalInput")
    ident16m = nc.dram_tensor("ident16m", [128, 2048], BF16, kind="ExternalInput")
    biasm = nc.dram_tensor("biasm", [128, 2], F32, kind="ExternalInput")
    dyW = nc.dram_tensor("dyW", [128, 288], BF16, kind="ExternalInput")
    dxW = nc.dram_tensor("dxW", [128, 288], BF16, kind="ExternalInput")
    mW = nc.dram_tensor("mW", [128, 288], F32, kind="ExternalInput")
    byW = nc.dram_tensor("byW", [128, 288], BF16, kind="ExternalInput")
    bxW = nc.dram_tensor("bxW", [128, 288], BF16, kind="ExternalInput")
    dyI = nc.dram_tensor("dyI", [128, 2304], BF16, kind="ExternalInput")
    dxI = nc.dram_tensor("dxI", [128, 2304], BF16, kind="ExternalInput")
    byI = nc.dram_tensor("byI", [128, 2304], BF16, kind="ExternalInput")
    bxI = nc.dram_tensor("bxI", [128, 2304], BF16, kind="ExternalInput")
    outT = nc.dram_tensor("out", [256, 4096], F32, kind="ExternalOutput")

    with ExitStack() as ctx:
        ec = ctx.enter_context

        # sbuf
        gdst = [ec(nc.sbuf_tensor(f"gdst{j}", [128, 8192], BF16)) for j in range(2)]
        w_sb = ec(nc.sbuf_tensor("wsb", [128, 4608], BF16))
        ident16 = ec(nc.sbuf_tensor("ident16", [128, 2048], BF16))
        bias_sb = ec(nc.sbuf_tensor("biassb", [128, 2], F32))
        Dd = [ec(nc.sbuf_tensor(f"Dd{j}", [128, 2048], BF16)) for j in range(2)]
        st_sb = [ec(nc.sbuf_tensor(f"stsb{j}", [128, 256], BF16)) for j in range(2)]
        out_sb = [ec(nc.sbuf_tensor(f"outsb{j}", [128, 1024], F32)) for j in range(2)]

        # W-layout field tiles [128, 288] + scratch
        fW_in = {n: ec(nc.sbuf_tensor(f"f_{n}", [128, 288],
                                      F32 if n == "mW" else BF16))
                 for n in ("dyW", "dxW", "mW", "byW", "bxW")}
        wc4b = ec(nc.sbuf_tensor("wc4b", [128, 1152], F32))
        wcor = [ec(nc.sbuf_tensor(f"wc{q}", [128, 288], F32)) for q in range(4)]
        tw = [ec(nc.sbuf_tensor(f"tw{j}", [128, 288], F32)) for j in range(7)]

        # I-layout tiles [128, 2304] for the gather-index chain
        fI_in = {n: ec(nc.sbuf_tensor(f"f_{n}", [128, 2304], BF16))
                 for n in ("dyI", "dxI", "byI", "bxI")}
        ti = [ec(nc.sbuf_tensor(f"ti{j}", [128, 2304], F32)) for j in range(4)]
        jw = ec(nc.sbuf_tensor("jw", [128, 2304], I16))

        # psum: full f32 banks
        psT = [ec(nc.psum_tensor(f"psT{j}", [128, 512], F32)) for j in range(2)]
        psE = [ec(nc.psum_tensor(f"psE{g}", [128, 512], F32)) for g in range(NG)]

        sem_ld = ec(nc.semaphore("sem_ld"))
        sem_prep = ec(nc.semaphore("sem_prep"))
        sem_prep2 = ec(nc.semaphore("sem_prep2"))
        sem_gat = ec(nc.semaphore("sem_gat"))
        sem_dve = ec(nc.semaphore("sem_dve"))
        sem_pet = ec(nc.semaphore("sem_pet"))
        sem_act = ec(nc.semaphore("sem_act"))
        sem_pee = ec(nc.semaphore("sem_pee"))
        sem_epi = ec(nc.semaphore("sem_epi"))
        sem_out = ec(nc.semaphore("sem_out"))

        # ---- AP helpers (flat element offsets) ----
        def sb(t, off, free, count=128):
            pstep = t.shape[1] if len(t.shape) == 2 else int(np.prod(t.shape[1:]))
            return AP(t, off, [[pstep, count], [1, free]])

        def scl(t, col):
            return AP(t, col, [[t.shape[1], 128], [1, 1]])

        # load order matters: I-layout index fields first (gate the J chain),
        # then W-layout fields + ident16 (corner weights / diag build), then
        # the einsum weights + bias.
        loads = []
        for n, src in (("dyI", dyI), ("dxI", dxI), ("byI", byI), ("bxI", bxI)):
            loads.append((sb(fI_in[n], 0, 2304), AP(src, 0, [[2304, 128], [1, 2304]])))
        for n, src in (("dyW", dyW), ("dxW", dxW), ("mW", mW), ("byW", byW),
                       ("bxW", bxW)):
            loads.append((sb(fW_in[n], 0, 288), AP(src, 0, [[288, 128], [1, 288]])))
        loads.append((sb(ident16, 0, 2048), AP(ident16m, 0, [[2048, 128], [1, 2048]])))
        loads.append((sb(bias_sb, 0, 2), AP(biasm, 0, [[2, 128], [1, 2]])))
        for kcb in range(18):
            loads.append((sb(w_sb, kcb * 256, 256),
                          AP(wT, kcb * 128 * 256, [[256, 128], [1, 256]])))
        n_loads = len(loads)
        LD_FI = 16 * 4      # I-layout fields loaded
        LD_FW = 16 * 10     # + W-layout fields + ident16

        glast = {}  # gi of last corner-mm step of each chunk -> chunk
        for c in range(NCH):
            glast[(c * K2 + (K2 - 1)) * NG + (NG - 1)] = c

        with nc.Block() as block:

            @block.sync
            def _(sync):
                for dst, src in loads:
                    sync.dma_start(dst, src).then_inc(sem_ld, 16)
                for c in range(NCH):
                    sync.wait_ge(sem_epi, 8 * (c + 1))
                    for ob in range(2):
                        dst = AP(outT, ob * 128 * 4096 + c * 512,
                                 [[4096, 128], [1, 512]])
                        src = sb(out_sb[c % 2], ob * 512, 512)
                        sync.dma_start(dst, src).then_inc(sem_out, 16)

            @block.gpsimd
            def _(gp):
                gp.load_library(mlp)
                in3_ap = AP(in3, 0, [[1024, R3], [1, 1024]])

                def trig(b):
                    # fire gather block b; its dst gdst[b%2] was last read by
                    # the corner-matmuls of iters {2(b-2), 2(b-2)+1}
                    gp.wait_ge(sem_prep2, b + 1)
                    if b >= 2:
                        gp.wait_ge(sem_pet, 8 * b - 8)
                    gp.trigger_dma(count=1, queue_num=b % 2)

                for b in range(NBLK):
                    # jw ready in 3 staged ranges
                    gp.wait_ge(sem_prep, 1 if b < 4 else (2 if b < 16 else 3))
                    dst = AP(gdst[b % 2], 0, [[8192, 128], [1024, 8], [1, 1024]])
                    idx = AP(jw, b * 64, [[2304, 128], [1, 64]])
                    gp.dma_gather(dst, in3_ap, idx, 1024, 1024, 1024,
                                  prepare_only=True, sem=sem_gat,
                                  queue_num=b % 2).then_inc(sem_prep2, 1)
                    if b >= 1:
                        trig(b - 1)
                trig(NBLK - 1)

            @block.vector
            def _(v):
                A = mybir.AluOpType

                class _G:
                    # auto-insert drain on same-engine RAW/WAR/WAW hazards
                    def __init__(self, eng):
                        self.e = eng
                        self.r = set()
                        self.w = set()

                    def drain(self):
                        self.e.drain()
                        self.r.clear()
                        self.w.clear()

                    def _run(self, fn, outs, ins, args, kwargs):
                        on = {o.tensor.name for o in outs}
                        innames = {a.tensor.name for a in ins
                                   if isinstance(a, AP)}
                        if (on & (self.r | self.w)) or (innames & self.w):
                            self.drain()
                        self.r |= innames
                        self.w |= on
                        return fn(*args, **kwargs)

                    def tensor_add(self, o, a, b):
                        return self._run(self.e.tensor_add, [o], [a, b],
                                         (o, a, b), {})

                    def tensor_sub(self, o, a, b):
                        return self._run(self.e.tensor_sub, [o], [a, b],
                                         (o, a, b), {})

                    def tensor_mul(self, o, a, b):
                        return self._run(self.e.tensor_mul, [o], [a, b],
                                         (o, a, b), {})

                    def tensor_tensor(self, o, a, b, op):
                        return self._run(self.e.tensor_tensor, [o], [a, b],
                                         (o, a, b, op), {})

                    def tensor_scalar(self, o, a, s1, s2, op0, op1):
                        return self._run(self.e.tensor_scalar, [o], [a],
                                         (o, a, s1, s2, op0, op1), {})

                    def tensor_single_scalar(self, o, a, s, op):
                        ins = [a] + ([s] if isinstance(s, AP) else [])
                        return self._run(self.e.tensor_single_scalar, [o], ins,
                                         (o, a, s, op), {})

                    def tensor_copy(self, o, a):
                        return self._run(self.e.tensor_copy, [o], [a],
                                         (o, a), {})

                    def scalar_tensor_tensor(self, o, a, s, b, op0, op1):
                        ins = [a, b] + ([s] if isinstance(s, AP) else [])
                        return self._run(self.e.scalar_tensor_tensor, [o], ins,
                                         (o, a, s, b, op0, op1), {})

                vg = _G(v)

                # ---- gather-index chain (I-layout [128, 2304]) ----
                # two passes: first 4 blocks' columns for a fast pipeline
                # start, then the rest
                def jchain(off, width):
                    s1, s2, s3, s4 = (sb(t, off, width) for t in ti)
                    vg.tensor_add(s1, sb(fI_in["dyI"], off, width),
                                  sb(fI_in["byI"], off, width))
                    vg.tensor_scalar(s2, s1, MAGIC, MAGIC, A.add, A.subtract)
                    vg.tensor_tensor(s3, s2, s1, A.is_gt)
                    vg.tensor_sub(s1, s2, s3)                       # ey
                    vg.tensor_scalar(s2, s1, -1.0, 63.0, A.max, A.min)  # py
                    vg.tensor_add(s3, sb(fI_in["dxI"], off, width),
                                  sb(fI_in["bxI"], off, width))
                    vg.tensor_scalar(s1, s3, MAGIC, MAGIC, A.add, A.subtract)
                    vg.tensor_tensor(s4, s1, s3, A.is_gt)
                    vg.tensor_sub(s1, s1, s4)                       # ex
                    vg.tensor_scalar(s3, s1, -1.0, 63.0, A.max, A.min)  # px
                    vg.tensor_scalar(s1, s3, float(PADLO), 0.0, A.add, A.add)
                    vg.scalar_tensor_tensor(s4, s2, 64.0, s1, A.mult, A.add)
                    vg.tensor_copy(AP(jw, off, [[2304, 128], [1, width]]),
                                   s4).then_inc(sem_prep)

                v.wait_ge(sem_ld, LD_FI)
                jchain(0, 256)
                v.wait_ge(sem_ld, LD_FW)

                # ---- corner-weight fields (W-layout [128, 288]) ----
                FW = 288
                t1, t2, t3, t4, t5, t6, t7 = (sb(t, 0, FW) for t in tw)
                dy = sb(fW_in["dyW"], 0, FW)
                dx = sb(fW_in["dxW"], 0, FW)
                mm = sb(fW_in["mW"], 0, FW)
                by = sb(fW_in["byW"], 0, FW)
                bx = sb(fW_in["bxW"], 0, FW)

                vg.tensor_add(t1, dy, by)                          # yA
                vg.tensor_scalar(t2, t1, MAGIC, MAGIC, A.add, A.subtract)
                vg.tensor_tensor(t3, t2, t1, A.is_gt)
                vg.tensor_sub(t4, t2, t3)                          # ey
                vg.tensor_sub(t5, t1, t4)                          # ly
                vg.tensor_scalar(t1, t5, -1.0, 1.0, A.mult, A.add)  # hy
                vg.tensor_single_scalar(t2, t4, 0.0, A.is_ge)
                vg.tensor_single_scalar(t3, t4, 63.0, A.is_le)
                vg.tensor_mul(t2, t2, t3)                          # vy0
                vg.tensor_single_scalar(t3, t4, -1.0, A.is_ge)
                vg.tensor_single_scalar(t6, t4, 62.0, A.is_le)
                vg.tensor_mul(t3, t3, t6)                          # vy1
                vg.tensor_mul(t2, t2, t1)
                vg.tensor_mul(t3, t3, t5)
                vg.tensor_mul(t2, t2, mm)                          # wy0m
                vg.tensor_mul(t3, t3, mm)                          # wy1m

                vg.tensor_add(t1, dx, bx)                          # xA
                vg.tensor_scalar(t4, t1, MAGIC, MAGIC, A.add, A.subtract)
                vg.tensor_tensor(t5, t4, t1, A.is_gt)
                vg.tensor_sub(t4, t4, t5)                          # ex
                vg.tensor_sub(t5, t1, t4)                          # lx
                vg.tensor_scalar(t1, t5, -1.0, 1.0, A.mult, A.add)  # hx
                vg.tensor_single_scalar(t6, t4, 0.0, A.is_ge)
                vg.tensor_single_scalar(t7, t4, 63.0, A.is_le)
                vg.tensor_mul(t6, t6, t7)                          # vx0
                vg.tensor_mul(t6, t6, t1)                          # cx0 = hx*vx0
                vg.tensor_single_scalar(t7, t4, -1.0, A.is_ge)
                vg.tensor_single_scalar(t4, t4, 62.0, A.is_le)
                vg.tensor_mul(t7, t7, t4)                          # vx1
                vg.tensor_mul(t7, t7, t5)                          # cx1 = lx*vx1

                vg.tensor_mul(sb(wcor[0], 0, FW), t2, t6)          # w00
                vg.tensor_mul(sb(wcor[1], 0, FW), t2, t7)          # w01
                vg.tensor_mul(sb(wcor[2], 0, FW), t3, t6)          # w10
                vg.tensor_mul(sb(wcor[3], 0, FW), t3, t7)          # w11

                # relayout wcor[q][p, k*32+c*4+g] -> wc4b[p, (c*9+k)*16+q*4+g]
                for q in range(4):
                    vg.tensor_copy(
                        AP(wc4b, q * 4, [[1152, 128], [144, 8], [16, 9], [1, 4]]),
                        AP(wcor[q], 0, [[288, 128], [4, 8], [32, 9], [1, 4]]))

                # ---- per-iteration diag build, interleaved with the J-chain
                # tail so the first gathers + first Dd are ready early ----
                iden_ap = AP(ident16, 0, [[2048, 128], [0, 16], [1, 128]])

                def dd(i):
                    # Dd[i%2][p, (q*4+g)*128 + j] = ident16[p, j] * w_{q,g}[p]
                    if i >= 2:
                        v.wait_ge(sem_pet, NG * (i - 1))
                    wsrc = AP(wc4b, i * 16, [[1152, 128], [1, 16], [0, 128]])
                    vg.tensor_tensor(
                        AP(Dd[i % 2], 0, [[2048, 128], [1, 2048]]),
                        iden_ap, wsrc, A.mult).then_inc(sem_dve)

                for i in range(4):
                    dd(i)
                jchain(256, 768)     # blocks 4-15
                for i in range(4, 12):
                    dd(i)
                jchain(1024, 1280)   # blocks 16-35
                for i in range(12, NITER):
                    dd(i)

            @block.tensor
            def _(te):
                te.wait_ge(sem_ld, 16 * n_loads)

                def emit_einsum(gg):
                    i2, g2 = divmod(gg, NG)
                    c2, k2 = divmod(i2, K2)
                    te.wait_ge(sem_act, gg + 1)
                    if k2 == 0 and c2 >= 1:
                        # psE[g2] bank reused across chunks; wait for the
                        # previous chunk's epilogue to finish reading it
                        te.wait_ge(sem_epi, 8 * c2)
                    last = None
                    for ob in range(2):
                        for cb in range(2):
                            lhs = AP(w_sb, (k2 * 2 + cb) * 256 + ob * 128,
                                     [[4608, 128], [1, 128]])
                            rhs = AP(st_sb[gg % 2], cb * 128, [[256, 128], [1, 128]])
                            dst = AP(psE[g2], ob * 128, [[512, 128], [1, 128]])
                            last = te.matmul(dst, lhs, rhs,
                                             start=(k2 == 0 and ob == 0
                                                    and cb == 0),
                                             stop=(k2 == K2 - 1 and ob == 1
                                                   and cb == 1))
                    last.then_inc(sem_pee)

                for i in range(NITER):
                    blk = i // 2
                    for g in range(NG):
                        gi = NG * i + g
                        te.wait_ge(sem_gat, 16 * (blk + 1))
                        te.wait_ge(sem_dve, i + 1)
                        if gi >= 2:
                            te.wait_ge(sem_act, gi - 1)
                        last = None
                        for q in range(4):
                            for cb in range(2):
                                lhs = AP(gdst[blk % 2],
                                         (i % 2) * 4096 + g * 1024 + q * 256
                                         + cb * 128,
                                         [[8192, 128], [1, 128]])
                                rhs = AP(Dd[i % 2], (q * 4 + g) * 128,
                                         [[2048, 128], [1, 128]])
                                dst = AP(psT[gi % 2], cb * 128,
                                         [[512, 128], [1, 128]])
                                last = te.matmul(dst, lhs, rhs,
                                                 start=(q == 0 and cb == 0),
                                                 stop=(q == 3 and cb == 1))
                        last.then_inc(sem_pet)
                        if gi >= 1:
                            emit_einsum(gi - 1)
                emit_einsum(NGI - 1)

            @block.scalar
            def _(sc):
                IDENT = mybir.ActivationFunctionType.Identity
                for gi in range(NGI):
                    sc.wait_ge(sem_pet, gi + 1)
                    if gi >= 2:
                        sc.wait_ge(sem_pee, gi - 1)
                    sc.activation(sb(st_sb[gi % 2], 0, 256),
                                  AP(psT[gi % 2], 0, [[512, 128], [1, 256]]),
                                  IDENT).then_inc(sem_act)
                    c = glast.get(gi)
                    if c is not None:
                        if c >= 2:
                            sc.wait_ge(sem_out, 32 * (c - 1))
                        for g2 in range(NG):
                            sc.wait_ge(sem_pee, (c * K2 + K2 - 1) * NG + g2 + 1)
                            for ob in range(2):
                                sc.activation(
                                    sb(out_sb[c % 2], ob * 512 + g2 * 128, 128),
                                    AP(psE[g2], ob * 128,
                                       [[512, 128], [1, 128]]),
                                    IDENT, bias=scl(bias_sb, ob),
                                ).then_inc(sem_epi)

    nc.compile()
    return nc


# ---------------------------------------------------------------------------
# host marshalling
# ---------------------------------------------------------------------------

def _to_W(f):
    # f [9, 4096] -> [128, 288]; fW[p%128, k*32 + p//128] = f[k, p]
    return np.ascontiguousarray(
        f.reshape(9, 32, 128).transpose(2, 0, 1).reshape(128, 288))


def _to_I(f):
    # f [9, 4096] -> wrapped [128, 2304]; fI[r, (c*9+k)*32+t] = f[k, c*512+t*16+r%16]
    # (c,k)-major so gather block b covers samples [1024b, 1024b+1024)
    a = f.reshape(9, 8, 32, 16).transpose(3, 1, 0, 2).reshape(16, 2304)
    return np.ascontiguousarray(np.tile(a, (8, 1)))


def marshal(inputs):
    import ml_dtypes
    np_bf16 = ml_dtypes.bfloat16

    inp = np.asarray(inputs["input"], np.float32)
    off = np.asarray(inputs["offset"], np.float32)
    msk = np.asarray(inputs["mask"], np.float32)
    wgt = np.asarray(inputs["weight"], np.float32)
    bias = np.asarray(inputs["bias"], np.float32)

    wT = np.ascontiguousarray(
        wgt.reshape(O, C, K2).transpose(2, 1, 0).reshape(2304, 256)).astype(np_bf16)
    biasm = np.ascontiguousarray(bias.reshape(2, 128).T)
    ident16 = np.zeros((128, 2048), np_bf16)
    for p in range(128):
        ident16[p, p::128] = 1.0

    ho = np.arange(HW, dtype=np.float32) // 64
    wo = np.arange(HW, dtype=np.float32) % 64
    ks = np.arange(K2, dtype=np.float32)
    by = ho[None, :] - 1.0 + (ks // 3)[:, None]
    bx = wo[None, :] - 1.0 + (ks % 3)[:, None]

    shared = {
        "wT": wT, "ident16m": ident16, "biasm": biasm,
        "byW": _to_W(by).astype(np_bf16), "bxW": _to_W(bx).astype(np_bf16),
        "byI": _to_I(by).astype(np_bf16), "bxI": _to_I(bx).astype(np_bf16),
    }

    in_maps = []
    for b in range(B):
        img = inp[b].transpose(1, 2, 0).reshape(HW, C)
        in2p = np.zeros((R2, C), np.float32)
        in2p[PADLO:PADLO + HW] = img
        in3 = np.zeros((R3, 1024), np.float32)
        n = HW + 2 * PADLO - 1  # 4225 usable rows
        in3[:n, 0:256] = in2p[0:n]
        in3[:n, 256:512] = in2p[1:n + 1]
        in3[:n, 512:768] = in2p[64:n + 64]
        in3[:n, 768:1024] = in2p[65:n + 65]

        # quantize offsets ONCE so the gather floor (I chain) and the corner
        # weights (W chain) see bit-identical values -> consistent corners
        off_y = off[b, 0::2].reshape(K2, HW).astype(np_bf16).astype(np.float32)
        off_x = off[b, 1::2].reshape(K2, HW).astype(np_bf16).astype(np.float32)
        im = {
            "in3": in3.astype(np_bf16),
            "dyW": _to_W(off_y).astype(np_bf16),
            "dxW": _to_W(off_x).astype(np_bf16),
            "mW": _to_W(msk[b].reshape(K2, HW)),
            "dyI": _to_I(off_y).astype(np_bf16),
            "dxI": _to_I(off_x).astype(np_bf16),
        }
        im.update(shared)
        in_maps.append(im)
    return in_maps


_NC_CACHE = {}


def _get_nc():
    if "nc" not in _NC_CACHE:
        _NC_CACHE["nc"] = build_nc()
    return _NC_CACHE["nc"]


def run(inputs, trace=False, **kw):
    nc = _get_nc()
    in_maps = marshal(inputs)
    res = bass_utils.run_bass_kernel_spmd(nc, in_maps, core_ids=list(range(B)),
                                          trace=trace, **kw)
    out = np.stack([r["out"].reshape(O, H, W) for r in res.results])
    return out.astype(np.float32), res


def kernel(**inputs):
    return run(inputs)[0]


# revision 26
# speedup vs baseline: 2.3903x; 1.0006x over previous
"""Modulated deformable conv v2 (B=8, C=O=256, H=W=64, 3x3) on 8 trn2 NeuronCores.

Strategy: data-parallel over batch (1 image per core). Per core:
  - host marshals the image into a "patch array" in3[j] = concat of padded
    [HW, C] rows (j, j+1, j+64, j+65) in bf16 so one contiguous 2KB DMA
    descriptor fetches the full 2x2 bilinear patch for all 256 channels of
    one sample.
  - device computes patch indices (floor/clip of offsets) on DVE, gpsimd
    dma_gather streams patch rows (HBM->SBUF) in 1024-sample blocks.
  - corner weighting + transpose + corner-sum are fused on the PE: for each
    128-position group, 8 matmuls  psT[ch, pos] += gdst_q[pos, ch]^T @
    diag(w_q[pos])  accumulate the 4 bilinear corners (weights fold
    bilinear frac * validity * modulation mask).  The diagonal tiles are
    rebuilt per iteration by one DVE op: Dd = ident16 * broadcast(w).
  - ACT copies PSUM->SBUF (cast bf16), PE runs the O x (C*K2) x HW einsum
    with PSUM accumulation, ACT folds the bias, HWDGE DMAs the output back.
"""

import numpy as np
from contextlib import ExitStack

import concourse.bacc as bacc
import concourse.bass as bass
import concourse.mybir as mybir
from concourse import bass_utils
from concourse.library_config import mlp

AP = bass.AP
F32 = mybir.dt.float32
BF16 = mybir.dt.bfloat16
I16 = mybir.dt.int16

# problem constants (hardcoded per contract)
B = 8
C = 256
O = 256
H = W = 64
HW = 4096
K2 = 9

NCH = 8           # spatial chunks
NG = 4            # 128-position groups per chunk
NITER = NCH * K2  # 72 (c, k) iterations, chunk-major
NGI = NITER * NG  # 288 (i, g) steps
NBLK = NITER // 2  # 36 gather blocks of 1024 samples

PADLO = 65        # leading pad rows in the padded [HW, C] image
R2 = 4292         # padded image rows
R3 = 4232         # patch-array rows (4225 used)
MAGIC = 12582912.0  # 1.5 * 2**23, round-to-nearest-even on f32


def build_nc():
    nc = bacc.Bacc("TRN2", detect_race_conditions=False,
                   dynamic_dma_scratch_size=32768, num_swdge_queues=2)

    in3 = nc.dram_tensor("in3", [R3, 1024], BF16, kind="ExternalInput")
    wT = nc.dram_tensor("wT", [2304, 256], BF16, kind="ExternalInput")
    ident16m = nc.dram_tensor("ident16m", [128, 2048], BF16, kind="ExternalInput")
    biasm = nc.dram_tensor("biasm", [128, 2], F32, kind="ExternalInput")
    dyW = nc.dram_tensor("dyW", [128, 288], BF16, kind="ExternalInput")
    dxW = nc.dram_tensor("dxW", [128, 288], BF16, kind="ExternalInput")
    mW = nc.dram_tensor("mW", [128, 288], F32, kind="ExternalInput")
    byW = nc.dram_tensor("byW", [128, 288], BF16, kind="ExternalInput")
    bxW = nc.dram_tensor("bxW", [128, 288], BF16, kind="ExternalInput")
    dyI = nc.dram_tensor("dyI", [128, 2304], BF16, kind="ExternalInput")
    dxI = nc.dram_tensor("dxI", [128, 2304], BF16, kind="ExternalInput")
    byI = nc.dram_tensor("byI", [128, 2304], BF16, kind="ExternalInput")
    bxI = nc.dram_tensor("bxI", [128, 2304], BF16, kind="ExternalInput")
    outT = nc.dram_tensor("out", [256, 4096], F32, kind="ExternalOutput")

    with ExitStack() as ctx:
        ec = ctx.enter_context

        # sbuf
        gdst = [ec(nc.sbuf_tensor(f"gdst{j}", [128, 8192], BF16)) for j in range(2)]
        w_sb = ec(nc.sbuf_tensor("wsb", [128, 4608], BF16))
        ident16 = ec(nc.sbuf_tensor("ident16", [128, 2048], BF16))
        bias_sb = ec(nc.sbuf_tensor("biassb", [128, 2], F32))
        Dd = [ec(nc.sbuf_tensor(f"Dd{j}", [128, 2048], BF16)) for j in range(2)]
        # st4[i%2][ch, g*256 + cb*128 + j] = sampled X[pos g*128+j, ch cb*128+a]
        st4 = [ec(nc.sbuf_tensor(f"st4_{j}", [128, 1024], BF16)) for j in range(2)]
        out_sb = [ec(nc.sbuf_tensor(f"outsb{j}", [128, 1024], F32)) for j in range(2)]

        # W-layout field tiles [128, 288] + scratch
        fW_in = {n: ec(nc.sbuf_tensor(f"f_{n}", [128, 288],
                                      F32 if n == "mW" else BF16))
                 for n in ("dyW", "dxW", "mW", "byW", "bxW")}
        wc4b = ec(nc.sbuf_tensor("wc4b", [128, 1152], F32))
        wcor = [ec(nc.sbuf_tensor(f"wc{q}", [128, 288], F32)) for q in range(4)]
        tw = [ec(nc.sbuf_tensor(f"tw{j}", [128, 288], F32)) for j in range(7)]

        # I-layout tiles [128, 2304] for the gather-index chain
        fI_in = {n: ec(nc.sbuf_tensor(f"f_{n}", [128, 2304], BF16))
                 for n in ("dyI", "dxI", "byI", "bxI")}
        ti = [ec(nc.sbuf_tensor(f"ti{j}", [128, 2304], F32)) for j in range(4)]
        jw = ec(nc.sbuf_tensor("jw", [128, 2304], I16))

        # psum: full f32 banks; psE2[ob][o, g*128+j] = out[ob*128+o, chunk pos]
        psT = [ec(nc.psum_tensor(f"psT{j}", [128, 512], F32)) for j in range(2)]
        psE2 = [ec(nc.psum_tensor(f"psE{ob}", [128, 512], F32)) for ob in range(2)]

        sem_ld = ec(nc.semaphore("sem_ld"))
        sem_prep = ec(nc.semaphore("sem_prep"))
        sem_prep2 = ec(nc.semaphore("sem_prep2"))
        sem_gat = ec(nc.semaphore("sem_gat"))
        sem_dve = ec(nc.semaphore("sem_dve"))
        sem_pet = ec(nc.semaphore("sem_pet"))
        sem_act = ec(nc.semaphore("sem_act"))
        sem_pee = ec(nc.semaphore("sem_pee"))
        sem_epi = ec(nc.semaphore("sem_epi"))
        sem_out = ec(nc.semaphore("sem_out"))

        # ---- AP helpers (flat element offsets) ----
        def sb(t, off, free, count=128):
            pstep = t.shape[1] if len(t.shape) == 2 else int(np.prod(t.shape[1:]))
            return AP(t, off, [[pstep, count], [1, free]])

        def scl(t, col):
            return AP(t, col, [[t.shape[1], 128], [1, 1]])

        # load order matters: I-layout index fields first (gate the J chain),
        # then W-layout fields + ident16 (corner weights / diag build), then
        # the einsum weights + bias.
        loads = []
        for n, src in (("dyI", dyI), ("dxI", dxI), ("byI", byI), ("bxI", bxI)):
            loads.append((sb(fI_in[n], 0, 2304), AP(src, 0, [[2304, 128], [1, 2304]])))
        for n, src in (("dyW", dyW), ("dxW", dxW), ("mW", mW), ("byW", byW),
                       ("bxW", bxW)):
            loads.append((sb(fW_in[n], 0, 288), AP(src, 0, [[288, 128], [1, 288]])))
        loads.append((sb(ident16, 0, 2048), AP(ident16m, 0, [[2048, 128], [1, 2048]])))
        loads.append((sb(bias_sb, 0, 2), AP(biasm, 0, [[2, 128], [1, 2]])))
        for kcb in range(18):
            loads.append((sb(w_sb, kcb * 256, 256),
                          AP(wT, kcb * 128 * 256, [[256, 128], [1, 256]])))
        n_loads = len(loads)
        LD_FI = 16 * 4      # I-layout fields loaded
        LD_FW = 16 * 10     # + W-layout fields + ident16

        glast = {}  # gi of last corner-mm step of each chunk -> chunk
        for c in range(NCH):
            glast[(c * K2 + (K2 - 1)) * NG + (NG - 1)] = c

        with nc.Block() as block:

            @block.sync
            def _(sync):
                for dst, src in loads:
                    sync.dma_start(dst, src).then_inc(sem_ld, 16)
                for c in range(NCH):
                    sync.wait_ge(sem_epi, 2 * (c + 1))
                    for ob in range(2):
                        dst = AP(outT, ob * 128 * 4096 + c * 512,
                                 [[4096, 128], [1, 512]])
                        src = sb(out_sb[c % 2], ob * 512, 512)
                        sync.dma_start(dst, src).then_inc(sem_out, 16)

            @block.gpsimd
            def _(gp):
                gp.load_library(mlp)
                in3_ap = AP(in3, 0, [[1024, R3], [1, 1024]])

                def trig(b):
                    # fire gather block b; its dst gdst[b%2] was last read by
                    # the corner-matmuls of iters {2(b-2), 2(b-2)+1}
                    gp.wait_ge(sem_prep2, b + 1)
                    if b >= 2:
                        gp.wait_ge(sem_pet, 8 * b - 8)
                    gp.trigger_dma(count=1, queue_num=b % 2)

                for b in range(NBLK):
                    # jw ready in 3 staged ranges
                    gp.wait_ge(sem_prep, 1 if b < 4 else (2 if b < 16 else 3))
                    dst = AP(gdst[b % 2], 0, [[8192, 128], [1024, 8], [1, 1024]])
                    idx = AP(jw, b * 64, [[2304, 128], [1, 64]])
                    gp.dma_gather(dst, in3_ap, idx, 1024, 1024, 1024,
                                  prepare_only=True, sem=sem_gat,
                                  queue_num=b % 2).then_inc(sem_prep2, 1)
                    if b >= 1:
                        trig(b - 1)
                trig(NBLK - 1)

            @block.vector
            def _(v):
                A = mybir.AluOpType

                class _G:
                    # auto-insert drain on same-engine RAW/WAR/WAW hazards
                    def __init__(self, eng):
                        self.e = eng
                        self.r = set()
                        self.w = set()

                    def drain(self):
                        self.e.drain()
                        self.r.clear()
                        self.w.clear()

                    def _run(self, fn, outs, ins, args, kwargs):
                        on = {o.tensor.name for o in outs}
                        innames = {a.tensor.name for a in ins
                                   if isinstance(a, AP)}
                        if (on & (self.r | self.w)) or (innames & self.w):
                            self.drain()
                        self.r |= innames
                        self.w |= on
                        return fn(*args, **kwargs)

                    def tensor_add(self, o, a, b):
                        return self._run(self.e.tensor_add, [o], [a, b],
                                         (o, a, b), {})

                    def tensor_sub(self, o, a, b):
                        return self._run(self.e.tensor_sub, [o], [a, b],
                                         (o, a, b), {})

                    def tensor_mul(self, o, a, b):
                        return self._run(self.e.tensor_mul, [o], [a, b],
                                         (o, a, b), {})

                    def tensor_tensor(self, o, a, b, op):
                        return self._run(self.e.tensor_tensor, [o], [a, b],
                                         (o, a, b, op), {})

                    def tensor_scalar(self, o, a, s1, s2, op0, op1):
                        return self._run(self.e.tensor_scalar, [o], [a],
                                         (o, a, s1, s2, op0, op1), {})

                    def tensor_single_scalar(self, o, a, s, op):
                        ins = [a] + ([s] if isinstance(s, AP) else [])
                        return self._run(self.e.tensor_single_scalar, [o], ins,
                                         (o, a, s, op), {})

                    def tensor_copy(self, o, a):
                        return self._run(self.e.tensor_copy, [o], [a],
                                         (o, a), {})

                    def scalar_tensor_tensor(self, o, a, s, b, op0, op1):
                        ins = [a, b] + ([s] if isinstance(s, AP) else [])
                        return self._run(self.e.scalar_tensor_tensor, [o], ins,
                                         (o, a, s, b, op0, op1), {})

                vg = _G(v)

                # ---- gather-index chain (I-layout [128, 2304]) ----
                # two passes: first 4 blocks' columns for a fast pipeline
                # start, then the rest
                def jchain(off, width):
                    s1, s2, s3, s4 = (sb(t, off, width) for t in ti)
                    vg.tensor_add(s1, sb(fI_in["dyI"], off, width),
                                  sb(fI_in["byI"], off, width))
                    vg.tensor_scalar(s2, s1, MAGIC, MAGIC, A.add, A.subtract)
                    vg.tensor_tensor(s3, s2, s1, A.is_gt)
                    vg.tensor_sub(s1, s2, s3)                       # ey
                    vg.tensor_scalar(s2, s1, -1.0, 63.0, A.max, A.min)  # py
                    vg.tensor_add(s3, sb(fI_in["dxI"], off, width),
                                  sb(fI_in["bxI"], off, width))
                    vg.tensor_scalar(s1, s3, MAGIC, MAGIC, A.add, A.subtract)
                    vg.tensor_tensor(s4, s1, s3, A.is_gt)
                    vg.tensor_sub(s1, s1, s4)                       # ex
                    vg.tensor_scalar(s3, s1, -1.0, 63.0, A.max, A.min)  # px
                    vg.tensor_scalar(s1, s3, float(PADLO), 0.0, A.add, A.add)
                    vg.scalar_tensor_tensor(s4, s2, 64.0, s1, A.mult, A.add)
                    vg.tensor_copy(AP(jw, off, [[2304, 128], [1, width]]),
                                   s4).then_inc(sem_prep)

                v.wait_ge(sem_ld, LD_FI)
                jchain(0, 256)
                v.wait_ge(sem_ld, LD_FW)

                # ---- corner-weight fields (W-layout [128, 288]) ----
                FW = 288
                t1, t2, t3, t4, t5, t6, t7 = (sb(t, 0, FW) for t in tw)
                dy = sb(fW_in["dyW"], 0, FW)
                dx = sb(fW_in["dxW"], 0, FW)
                mm = sb(fW_in["mW"], 0, FW)
                by = sb(fW_in["byW"], 0, FW)
                bx = sb(fW_in["bxW"], 0, FW)

                vg.tensor_add(t1, dy, by)                          # yA
                vg.tensor_scalar(t2, t1, MAGIC, MAGIC, A.add, A.subtract)
                vg.tensor_tensor(t3, t2, t1, A.is_gt)
                vg.tensor_sub(t4, t2, t3)                          # ey
                vg.tensor_sub(t5, t1, t4)                          # ly
                vg.tensor_scalar(t1, t5, -1.0, 1.0, A.mult, A.add)  # hy
                vg.tensor_single_scalar(t2, t4, 0.0, A.is_ge)
                vg.tensor_single_scalar(t3, t4, 63.0, A.is_le)
                vg.tensor_mul(t2, t2, t3)                          # vy0
                vg.tensor_single_scalar(t3, t4, -1.0, A.is_ge)
                vg.tensor_single_scalar(t6, t4, 62.0, A.is_le)
                vg.tensor_mul(t3, t3, t6)                          # vy1
                vg.tensor_mul(t2, t2, t1)
                vg.tensor_mul(t3, t3, t5)
                vg.tensor_mul(t2, t2, mm)                          # wy0m
                vg.tensor_mul(t3, t3, mm)                          # wy1m

                vg.tensor_add(t1, dx, bx)                          # xA
                vg.tensor_scalar(t4, t1, MAGIC, MAGIC, A.add, A.subtract)
                vg.tensor_tensor(t5, t4, t1, A.is_gt)
                vg.tensor_sub(t4, t4, t5)                          # ex
                vg.tensor_sub(t5, t1, t4)                          # lx
                vg.tensor_scalar(t1, t5, -1.0, 1.0, A.mult, A.add)  # hx
                vg.tensor_single_scalar(t6, t4, 0.0, A.is_ge)
                vg.tensor_single_scalar(t7, t4, 63.0, A.is_le)
                vg.tensor_mul(t6, t6, t7)                          # vx0
                vg.tensor_mul(t6, t6, t1)                          # cx0 = hx*vx0
                vg.tensor_single_scalar(t7, t4, -1.0, A.is_ge)
                vg.tensor_single_scalar(t4, t4, 62.0, A.is_le)
                vg.tensor_mul(t7, t7, t4)                          # vx1
                vg.tensor_mul(t7, t7, t5)                          # cx1 = lx*vx1

                vg.tensor_mul(sb(wcor[0], 0, FW), t2, t6)          # w00
                vg.tensor_mul(sb(wcor[1], 0, FW), t2, t7)          # w01
                vg.tensor_mul(sb(wcor[2], 0, FW), t3, t6)          # w10
                vg.tensor_mul(sb(wcor[3], 0, FW), t3, t7)          # w11

                # relayout wcor[q][p, k*32+c*4+g] -> wc4b[p, (c*9+k)*16+q*4+g]
                for q in range(4):
                    vg.tensor_copy(
                        AP(wc4b, q * 4, [[1152, 128], [144, 8], [16, 9], [1, 4]]),
                        AP(wcor[q], 0, [[288, 128], [4, 8], [32, 9], [1, 4]]))

                # ---- per-iteration diag build, interleaved with the J-chain
                # tail so the first gathers + first Dd are ready early ----
                iden_ap = AP(ident16, 0, [[2048, 128], [0, 16], [1, 128]])

                def dd(i):
                    # Dd[i%2][p, (q*4+g)*128 + j] = ident16[p, j] * w_{q,g}[p]
                    if i >= 2:
                        v.wait_ge(sem_pet, NG * (i - 1))
                    wsrc = AP(wc4b, i * 16, [[1152, 128], [1, 16], [0, 128]])
                    vg.tensor_tensor(
                        AP(Dd[i % 2], 0, [[2048, 128], [1, 2048]]),
                        iden_ap, wsrc, A.mult).then_inc(sem_dve)

                for i in range(4):
                    dd(i)
                jchain(256, 768)     # blocks 4-15
                for i in range(4, 20):
                    dd(i)
                jchain(1024, 1280)   # blocks 16-35
                for i in range(20, NITER):
                    dd(i)

            @block.tensor
            def _(te):
                te.wait_ge(sem_ld, 16 * n_loads)

                def emit_einsum(i2):
                    # one 512-moving matmul per (ob, cb): all 4 groups of
                    # iter i2 share the same stationary weight tile
                    c2, k2 = divmod(i2, K2)
                    te.wait_ge(sem_act, NG * (i2 + 1))
                    if k2 == 0 and c2 >= 1:
                        # psE2 banks reused across chunks; wait for the
                        # previous chunk's epilogue to finish reading them
                        te.wait_ge(sem_epi, 2 * c2)
                    last = None
                    for ob in range(2):
                        for cb in range(2):
                            lhs = AP(w_sb, (k2 * 2 + cb) * 256 + ob * 128,
                                     [[4608, 128], [1, 128]])
                            rhs = AP(st4[i2 % 2], cb * 128,
                                     [[1024, 128], [256, 4], [1, 128]])
                            dst = AP(psE2[ob], 0, [[512, 128], [1, 512]])
                            last = te.matmul(dst, lhs, rhs,
                                             start=(k2 == 0 and cb == 0),
                                             stop=(k2 == K2 - 1 and cb == 1))
                    last.then_inc(sem_pee)

                for i in range(NITER):
                    blk = i // 2
                    for g in range(NG):
                        gi = NG * i + g
                        te.wait_ge(sem_gat, 16 * (blk + 1))
                        te.wait_ge(sem_dve, i + 1)
                        if gi >= 2:
                            te.wait_ge(sem_act, gi - 1)
                        last = None
                        for q in range(4):
                            for cb in range(2):
                                lhs = AP(gdst[blk % 2],
                                         (i % 2) * 4096 + g * 1024 + q * 256
                                         + cb * 128,
                                         [[8192, 128], [1, 128]])
                                rhs = AP(Dd[i % 2], (q * 4 + g) * 128,
                                         [[2048, 128], [1, 128]])
                                dst = AP(psT[gi % 2], cb * 128,
                                         [[512, 128], [1, 128]])
                                last = te.matmul(dst, lhs, rhs,
                                                 start=(q == 0 and cb == 0),
                                                 stop=(q == 3 and cb == 1))
                        last.then_inc(sem_pet)
                        if g == 1 and i >= 1:
                            emit_einsum(i - 1)
                emit_einsum(NITER - 1)

            @block.scalar
            def _(sc):
                IDENT = mybir.ActivationFunctionType.Identity
                for gi in range(NGI):
                    i, g = divmod(gi, NG)
                    sc.wait_ge(sem_pet, gi + 1)
                    if i >= 2:
                        # st4[i%2] last read by einsum(i-2)
                        sc.wait_ge(sem_pee, i - 1)
                    sc.activation(sb(st4[i % 2], g * 256, 256),
                                  AP(psT[gi % 2], 0, [[512, 128], [1, 256]]),
                                  IDENT).then_inc(sem_act)
                    c = glast.get(gi)
                    if c is not None:
                        if c >= 2:
                            sc.wait_ge(sem_out, 32 * (c - 1))
                        sc.wait_ge(sem_pee, c * K2 + K2)
                        for ob in range(2):
                            sc.activation(
                                sb(out_sb[c % 2], ob * 512, 512),
                                AP(psE2[ob], 0, [[512, 128], [1, 512]]),
                                IDENT, bias=scl(bias_sb, ob),
                            ).then_inc(sem_epi)

    nc.compile()
    return nc


# ---------------------------------------------------------------------------
# host marshalling
# ---------------------------------------------------------------------------

def _to_W(f):
    # f [9, 4096] -> [128, 288]; fW[p%128, k*32 + p//128] = f[k, p]
    return np.ascontiguousarray(
        f.reshape(9, 32, 128).transpose(2, 0, 1).reshape(128, 288))


def _to_I(f):
    # f [9, 4096] -> wrapped [128, 2304]; fI[r, (c*9+k)*32+t] = f[k, c*512+t*16+r%16]
    # (c,k)-major so gather block b covers samples [1024b, 1024b+1024)
    a = f.reshape(9, 8, 32, 16).transpose(3, 1, 0, 2).reshape(16, 2304)
    return np.ascontiguousarray(np.tile(a, (8, 1)))


def marshal(inputs):
    import ml_dtypes
    np_bf16 = ml_dtypes.bfloat16

    inp = np.asarray(inputs["input"], np.float32)
    off = np.asarray(inputs["offset"], np.float32)
    msk = np.asarray(inputs["mask"], np.float32)
    wgt = np.asarray(inputs["weight"], np.float32)
    bias = np.asarray(inputs["bias"], np.float32)

    wT = np.ascontiguousarray(
        wgt.reshape(O, C, K2).transpose(2, 1, 0).reshape(2304, 256)).astype(np_bf16)
    biasm = np.ascontiguousarray(bias.reshape(2, 128).T)
    ident16 = np.zeros((128, 2048), np_bf16)
    for p in range(128):
        ident16[p, p::128] = 1.0

    ho = np.arange(HW, dtype=np.float32) // 64
    wo = np.arange(HW, dtype=np.float32) % 64
    ks = np.arange(K2, dtype=np.float32)
    by = ho[None, :] - 1.0 + (ks // 3)[:, None]
    bx = wo[None, :] - 1.0 + (ks % 3)[:, None]

    shared = {
        "wT": wT, "ident16m": ident16, "biasm": biasm,
        "byW": _to_W(by).astype(np_bf16), "bxW": _to_W(bx).astype(np_bf16),
        "byI": _to_I(by).astype(np_bf16), "bxI": _to_I(bx).astype(np_bf16),
    }

    in_maps = []
    for b in range(B):
        img = inp[b].transpose(1, 2, 0).reshape(HW, C)
        in2p = np.zeros((R2, C), np.float32)
        in2p[PADLO:PADLO + HW] = img
        in3 = np.zeros((R3, 1024), np.float32)
        n = HW + 2 * PADLO - 1  # 4225 usable rows
        in3[:n, 0:256] = in2p[0:n]
        in3[:n, 256:512] = in2p[1:n + 1]
        in3[:n, 512:768] = in2p[64:n + 64]
        in3[:n, 768:1024] = in2p[65:n + 65]

        # quantize offsets ONCE so the gather floor (I chain) and the corner
        # weights (W chain) see bit-identical values -> consistent corners
        off_y = off[b, 0::2].reshape(K2, HW).astype(np_bf16).astype(np.float32)
        off_x = off[b, 1::2].reshape(K2, HW).astype(np_bf16).astype(np.float32)
        im = {
            "in3": in3.astype(np_bf16),
            "dyW": _to_W(off_y).astype(np_bf16),
            "dxW": _to_W(off_x).astype(np_bf16),
            "mW": _to_W(msk[b].reshape(K2, HW)),
            "dyI": _to_I(off_y).astype(np_bf16),
            "dxI": _to_I(off_x).astype(np_bf16),
        }
        im.update(shared)
        in_maps.append(im)
    return in_maps


_NC_CACHE = {}


def _get_nc():
    if "nc" not in _NC_CACHE:
        _NC_CACHE["nc"] = build_nc()
    return _NC_CACHE["nc"]


def run(inputs, trace=False, **kw):
    nc = _get_nc()
    in_maps = marshal(inputs)
    res = bass_utils.run_bass_kernel_spmd(nc, in_maps, core_ids=list(range(B)),
                                          trace=trace, **kw)
    out = np.stack([r["out"].reshape(O, H, W) for r in res.results])
    return out.astype(np.float32), res


def kernel(**inputs):
    return run(inputs)[0]
